# revision 42
# baseline (speedup 1.0000x reference)
"""Trainium2 Bass kernel for nn_Ansatz_44573170598544.

QMC ansatz: per-walker Jastrow + 2-qubit PQC backflow + two 8x8 complex
Slater log-determinants. Pure data parallel: 32768 walkers over 8 cores.

Host-precomputed transforms (validated vs reference in numpy):
  * PQC z = f^T Q f, f = kron of (cos,sin) of 4 half-angles;
    Q = V diag(lam) V^T -> z = sum_i lam_i (V^T f)_i^2: TensorE
    block-diag matmuls in feature-major layout via PE transposes.
  * Slater E[n,m] = exp(i r_n.k_m); kvec 0 is (0,0) -> column 0 all-ones
    -> LU step 0 is a row subtraction; then batched no-pivot LU on 7x7
    via stride-0 broadcast APs (8192 matrices/core).
  * logdet = 0.5*Ln(|det|^2) + i*atan2(Im,Re) per spin (matches
    log(sign)+logabs of slogdet; no branch-cut wrapping).

Layouts (per core, 4096 walkers):
  particle-major planes [128, 512]: particle m = part*512 + col.
  matrix (w,s) -> part p = (w*16+8*s)//512, chunk tc = ((w*16+8*s)%512)//8
    i.e. w = p*32 + tc//2, s = tc%2; its 8 particles are columns
    tc*8..tc*8+7 of partition p.  Walker w -> (p, tw=tc//2).
  Jastrow & output use the same walker mapping (p, tw).

Scheduling notes (engines have in-order queues; emission order matters):
  * per half: both q PE-feed groups, then that half's phasor/LU --
    LU0 overlaps hf1's PE/ACT feed (measured 18us faster than emitting
    all four feed groups before any phasor work).
  * jastrow differences/abs-chain fill the early DVE window; sqrt +
    jastrow polynomial run on the tail-idle ACT (Identity scale/bias
    passes) + Pool (tensor muls).
  * lambda-contraction matmuls use bf16 gsq weights (fp32 weights cost
    2 half-rate PE passes each).
  * LU rank-1 updates: lre/lim expanded to packed bf16 [t,r,r] tiles by
    ACT copies so the DVE products run in the 2x (2-byte) mode; the A
    accumulation stays f32.  Pivots stay on the A diagonal (never
    overwritten); logdet = sum Ln|p_k|^2 + i*sum atan2 over the 7 pivots,
    batched wide at the end, phase wrapped to (-pi, pi].
  * output written interleaved [128,32,2] -> one contiguous DMA (the
    strided re/im pair DMA cost ~70us in 4-byte descriptors).
"""
import sys
import numpy as np

sys.path.insert(0, "/opt/trn_rl_repo")

from concourse import bass, mybir, tile  # noqa: E402
from concourse.bass_utils import run_bass_kernel_spmd  # noqa: E402

F32 = mybir.dt.float32
BF16 = mybir.dt.bfloat16
ALU = mybir.AluOpType
ACT = mybir.ActivationFunctionType
AXL = mybir.AxisListType

NCORES = 8
B = 32768
BLOC = B // NCORES          # 4096 walkers/core
NW = BLOC // 128            # 32 walkers per partition
M = BLOC * 16               # 65536 particles/core
MC = M // 128               # 512 particle columns
MCH = MC // 2               # pqc/LU half: 256 cols = 32 mats/part
BSH = 32                    # matrices per partition per half
JCH = 2
JB = NW // JCH              # 16 walkers per jastrow chunk
PIH = float(np.pi / 2)
PI = float(np.pi)

C_JP = 0
C_SC = [6, 18]
C_BC = [10, 22]
C_BSN = [14, 26]
C_KX = 32
C_KY = 39
C_MHALF = 46
C_PIHC = 47
C_MONE = 48
C_TPI = 49
C_MTPI = 50
C_HALF = 51
C_MTWO = 52
C_ONE = 53
NCONS = 54
WQ = 144                    # per-q weights: W_both 128 (rank-8, j-fused), RL_both 16
NWV = 2 * WQ


def _host_precompute(inputs):
    def rz(t):
        e = np.exp(-0.5j * t)
        return np.diag([e, np.conj(e)])

    def ry(t):
        c, s = np.cos(0.5 * t), np.sin(0.5 * t)
        return np.array([[c, -s], [s, c]], complex)

    def euler(p):
        return rz(p[2]) @ ry(p[1]) @ rz(p[0])

    def entangler(t):
        I4 = np.eye(4, dtype=complex)
        dzz = np.array([1., -1., -1., 1.])
        XX = np.array([[0, 0, 0, 1], [0, 0, 1, 0], [0, 1, 0, 0],
                       [1, 0, 0, 0]], complex)
        YY = np.array([[0, 0, 0, -1], [0, 0, 1, 0], [0, 1, 0, 0],
                       [-1, 0, 0, 0]], complex)
        rzz = lambda a: np.diag(np.exp(-0.5j * a * dzz))
        rxx = np.cos(0.5 * t[1]) * I4 - 1j * np.sin(0.5 * t[1]) * XX
        ryy = np.cos(0.5 * t[2]) * I4 - 1j * np.sin(0.5 * t[2]) * YY
        return rzz(t[3]) @ ryy @ rxx @ rzz(t[0])

    cons = np.zeros((128, NCONS), np.float32)
    wv = np.zeros((128, NWV), np.float32)
    cons[:, C_JP:C_JP + 6] = np.asarray(inputs["jastrow_param"], np.float64)
    X = np.array([[0, 1], [1, 0]], complex)
    I2 = np.eye(2, dtype=complex)
    for q in range(2):
        sq = np.asarray(inputs["param_single_qubit"][q], np.float64)
        tq = np.asarray(inputs["param_two_qubit"][q], np.float64)
        enc = np.asarray(inputs["param_encoding"][q], np.float64)
        enc_b = np.asarray(inputs["param_encoding_bias"][q], np.float64)
        U1 = (np.kron(euler(sq[0, 2]), euler(sq[0, 3])) @ entangler(tq[0])
              @ np.kron(euler(sq[0, 0]), euler(sq[0, 1])))
        U2 = (np.kron(euler(sq[1, 2]), euler(sq[1, 3])) @ entangler(tq[1])
              @ np.kron(euler(sq[1, 0]), euler(sq[1, 1])))
        D1 = np.diag([1, -1j, -1j, -1]).astype(complex)
        M1 = U1 @ D1
        KS = [np.kron(I2, I2), np.kron(I2, -1j * X),
              np.kron(-1j * X, I2), np.kron(-1j * X, -1j * X)]
        T = np.zeros((4, 16), complex)
        for i1 in range(2):
            for i2 in range(2):
                e12 = np.zeros(4)
                e12[2 * i1 + i2] = 1.0
                base = M1 @ e12
                for i3 in range(2):
                    for i4 in range(2):
                        T[:, 8 * i1 + 4 * i2 + 2 * i3 + i4] = \
                            U2 @ KS[2 * i3 + i4] @ base
        Z0 = np.diag([1., 1., -1., -1.]).astype(complex)
        Z1 = np.diag([1., -1., 1., -1.]).astype(complex)
        # Q has exact rank 8; keep the 8 nonzero eigenpairs and fuse the
        # two observables (j) into one weight block: out part = (j,a2,c).
        Wb = np.zeros((128, 128), np.float32)
        RLb = np.zeros((128, 16), np.float32)
        for j, Z in enumerate((Z0, Z1)):
            Q = np.real(T.conj().T @ Z @ T)
            Q = 0.5 * (Q + Q.T)
            lam, V = np.linalg.eigh(Q)
            idx8 = np.argsort(np.abs(lam))[-8:]
            V8 = V[:, idx8].astype(np.float32)
            lam8 = lam[idx8].astype(np.float32)
            for c in range(8):
                rows = np.arange(16) * 8 + c
                colsb = j * 64 + np.arange(8) * 8 + c
                Wb[np.ix_(rows, colsb)] = V8
                RLb[colsb, j * 8 + c] = lam8
        wv[:, q * WQ: q * WQ + 128] = Wb
        wv[:, q * WQ + 128: q * WQ + 144] = RLb
        scale = np.array([enc[0, 0], enc[0, 1], enc[1, 0], enc[1, 1]]) * np.pi
        bias = 0.5 * np.array([enc_b[0, 0], enc_b[0, 1],
                               enc_b[1, 0], enc_b[1, 1]])
        cons[:, C_SC[q]:C_SC[q] + 4] = scale
        cons[:, C_BC[q]:C_BC[q] + 4] = bias + PIH
        cons[:, C_BSN[q]:C_BSN[q] + 4] = bias
    kv = np.asarray(inputs["kvecs"], np.float64)
    assert abs(kv[0]).max() < 1e-6, "kernel assumes kvecs[0] == 0"
    cons[:, C_KX:C_KX + 7] = kv[1:8, 0]
    cons[:, C_KY:C_KY + 7] = kv[1:8, 1]
    cons[:, C_MHALF] = -0.5
    cons[:, C_PIHC] = PIH
    cons[:, C_MONE] = -1.0
    cons[:, C_TPI] = 2 * PI
    cons[:, C_MTPI] = -2 * PI
    cons[:, C_HALF] = 0.5
    cons[:, C_MTWO] = -2.0
    cons[:, C_ONE] = 1.0
    pc = np.asarray(inputs["param_classical"], np.float64)
    pcv = np.zeros((128, 4), np.float32)
    pcv[:, 0:2] = pc[0]
    pcv[:, 2:4] = pc[1]
    ident = np.eye(128, dtype=np.float32)
    return cons, wv, ident, pcv


def build(gpat=None, debug=False, loop_n=0, trace_sim=False):
    if gpat is None:
        gpat = ((-1, 0), (0, -1), (0, 1), (1, 0),
                (-1, -1), (-1, 1), (1, -1))
    nc = bass.Bass()
    x_d = nc.declare_dram_parameter("x", [BLOC, 32], F32, isOutput=False)
    NKC = NCONS + NWV + 128 + 4
    kc_d = nc.declare_dram_parameter("kc", [128, NKC], F32, isOutput=False)
    out_d = nc.declare_dram_parameter("out", [BLOC, 2], F32, isOutput=True)
    if debug:
        dbg_jas = nc.declare_dram_parameter("dbg_jas", [128, NW], F32,
                                            isOutput=True)
        dbg_z = nc.declare_dram_parameter("dbg_z", [128, 4, MC], F32,
                                          isOutput=True)
        dbg_E = nc.declare_dram_parameter("dbg_E", [128, 2, BSH, 8, 7], F32,
                                          isOutput=True)
        dbg_det = nc.declare_dram_parameter("dbg_det", [128, 4, BSH], F32,
                                            isOutput=True)

    xflat = x_d[:].rearrange("b c -> (b c)")

    _tc_holder = {}
    with tile.TileContext(nc, trace_sim=trace_sim) as tc:
        _tc_holder['tc'] = tc
        with (
            tc.tile_pool(name="const", bufs=1) as cpool,
            tc.tile_pool(name="pers", bufs=1) as pers,
            tc.tile_pool(name="jt", bufs=1) as jt,
            tc.tile_pool(name="pt", bufs=1) as pt,
            tc.tile_pool(name="gt", bufs=4) as gt,
            tc.tile_pool(name="et", bufs=2) as et,
            tc.tile_pool(name="lt", bufs=2) as lt,
            tc.tile_pool(name="st", bufs=2) as st,
            tc.tile_pool(name="ps_t", bufs=3, space="PSUM") as ps_t,
            tc.tile_pool(name="ps_g", bufs=3, space="PSUM") as ps_g,
            tc.tile_pool(name="ps_w", bufs=1, space="PSUM") as ps_w,
        ):
            kc = cpool.tile([128, NKC], F32, tag="kc")
            nc.sync.dma_start(kc[:], kc_d[:])
            cons = kc[:, 0:NCONS]
            wvt = kc[:, NCONS:NCONS + NWV]
            ident = kc[:, NCONS + NWV:NCONS + NWV + 128]
            pcv = kc[:, NCONS + NWV + 128:NCONS + NWV + 132]

            wvb = cpool.tile([128, NWV], BF16, tag="wvb")
            nc.scalar.copy(wvb[:], kc[:, NCONS:NCONS + NWV])
            identb = cpool.tile([128, 128], BF16, tag="identb")
            nc.scalar.copy(identb[:], ident)

            def cc(i):
                return cons[:, i:i + 1]

            # =============== Jastrow (walker (p,tw) mapping) ===========
            xin = pers.tile([128, NW, 32], F32, tag="xin")
            xdv = xflat.rearrange("(p tw c) -> p tw c", p=128, tw=NW, c=32)
            # split the input DMA by walker halves: half-0 compute
            # (jastrow chunk 0, hf0 trig) starts after the first half lands
            nc.sync.dma_start(xin[:, 0:NW // 2, :], xdv[:, 0:NW // 2, :])
            nc.sync.dma_start(xin[:, NW // 2:NW, :], xdv[:, NW // 2:NW, :])
            xall = xin[:].rearrange("p tw c -> p (tw c)").rearrange(
                "p (cc d) -> p cc d", d=2)
            xsep = pers.tile([128, 2, MC], F32, tag="xsep")
            for hx in range(2):
                hs = slice(hx * MCH, (hx + 1) * MCH)
                nc.vector.tensor_copy(xsep[:, 0, hs], xall[:, hs, 0])
                nc.vector.tensor_copy(xsep[:, 1, hs], xall[:, hs, 1])
            # ScalarE warm-ups: observe each DMA queue once so no real ACT
            # instruction ever needs two semaphore waits (ISA limit is 1).
            wsc1 = cpool.tile([128, 1], F32, tag="wsc1")
            wsc2 = cpool.tile([128, 1], F32, tag="wsc2")
            nc.scalar.activation(wsc1[:], kc[:, 0:1], ACT.Copy)
            nc.scalar.activation(wsc2[:], xin[:, 0, 0:1], ACT.Copy)
            wps = ps_w.tile([8, 8], F32, tag="wps")
            nc.tensor.transpose(wps[:], ident[0:8, 0:8], ident[0:8, 0:8])
            import contextlib
            _lcm = tc.For_i(0, loop_n, 1) if loop_n else \
                contextlib.nullcontext()
            _lcm.__enter__()
            # =============== PQC backflow -> zplm[q][:,j,:] ============
            zplm = [pers.tile([128, 2, MC], F32, tag=f"zplm{q}",
                              name=f"zplm{q}") for q in range(2)]
            zpl = [[zplm[q][:, j] for j in range(2)] for q in range(2)]


            # PQC frontend for all (hf,q) first: trig (ACT) + feature
            # products (DVE) so the PE transpose/matmul pipeline starts
            # immediately; jastrow then fills ACT/DVE/Pool gaps.
            fts = {}

            def _frontend(hf):
                c0 = hf * MCH
                for q in range(2):
                    trig = pt.tile([128, 8, MCH], BF16, tag="trig")
                    for j in range(4):
                        coord = xsep[:, j % 2, c0:c0 + MCH]
                        nc.scalar.activation(trig[:, 2 * j, :], coord,
                                             ACT.Sin, bias=cc(C_BC[q] + j),
                                             scale=cc(C_SC[q] + j))
                        nc.scalar.activation(trig[:, 2 * j + 1, :], coord,
                                             ACT.Sin, bias=cc(C_BSN[q] + j),
                                             scale=cc(C_SC[q] + j))
                    u = pt.tile([128, 2, 2, MCH], BF16, tag="u")
                    nc.vector.tensor_mul(
                        u[:],
                        trig[:, 0:2, :].unsqueeze(2).broadcast_to(
                            (128, 2, 2, MCH)),
                        trig[:, 2:4, :].unsqueeze(1).broadcast_to(
                            (128, 2, 2, MCH)))
                    v = pt.tile([128, 2, 2, MCH], BF16, tag="v")
                    nc.vector.tensor_mul(
                        v[:],
                        trig[:, 4:6, :].unsqueeze(2).broadcast_to(
                            (128, 2, 2, MCH)),
                        trig[:, 6:8, :].unsqueeze(1).broadcast_to(
                            (128, 2, 2, MCH)))
                    f = pt.tile([128, MCH // 8, 16, 8], BF16,
                                tag=f"f{hf}{q}")
                    fo = f[:].rearrange("p t (a b) c -> p a b t c", a=4)
                    nc.vector.tensor_mul(
                        fo,
                        u[:].rearrange("p a b (t c) -> p (a b) t c", c=8)
                            .unsqueeze(2).broadcast_to(
                                (128, 4, 4, MCH // 8, 8)),
                        v[:].rearrange("p a b (t c) -> p (a b) t c", c=8)
                            .unsqueeze(1).broadcast_to(
                                (128, 4, 4, MCH // 8, 8)))
                    fts[(hf, q)] = f

            _frontend(0)

            # ===== Jastrow front: fused pair differences only (DVE).
            # The |d|/min-image/poly chain runs post-feed on the
            # tail-idle ACT (see below).
            jas = pers.tile([128, NW], F32, tag="jas")
            js = pers.tile([128, NW, 120], BF16, tag="js")
            jd = jt.tile([128, NW, 240], BF16, tag="jd")
            off = 0
            for o in range(1, 16):
                Lg = 32 - 2 * o
                nc.vector.tensor_sub(jd[:, :, off:off + Lg],
                                     xin[:, :, 0:Lg],
                                     xin[:, :, 2 * o:32])
                off += Lg
            outri = pers.tile([128, NW, 2], F32, tag="outri")
            prs = pers.tile([128, 2, BSH, 7], F32, tag="prs")
            pis = pers.tile([128, 2, BSH, 7], F32, tag="pis")
            # PE feed for all four (hf,q) groups first — keeps the
            # in-order ACT queue free of phasor ops that would stall it.
            for hf in range(2):
                for q in range(2):
                    f = fts[(hf, q)]
                    for gl in range(8):
                        grp = hf * 8 + gl
                        ftp = ps_t.tile([128, 512], BF16, tag="ftp")
                        for gi in range(4):
                            ti = gl * 4 + gi
                            nc.tensor.transpose(
                                ftp[:, gi * 128:(gi + 1) * 128],
                                f[:, ti].rearrange("p a c -> p (a c)"),
                                identb[:])
                        ftr = gt.tile([128, 512], BF16, tag="ftr")
                        nc.scalar.copy(ftr[:], ftp[:])
                        gp = ps_g.tile([128, 512], F32, tag="gp")
                        ztp = ps_w.tile([128, 4, 2, 8], F32, tag="ztp")
                        nc.tensor.matmul(
                            gp[:],
                            wvb[:, q * WQ:q * WQ + 128],
                            ftr[:])
                        gsq = gt.tile([128, 512], BF16, tag="gsq")
                        nc.scalar.activation(gsq[:], gp[:], ACT.Square)
                        for gi in range(4):
                            nc.tensor.matmul(
                                ztp[:, gi, :, :],
                                gsq[:, gi * 128:(gi + 1) * 128],
                                wvb[:, q * WQ + 128:q * WQ + 144])
                        nc.scalar.copy(
                            zplm[q][:, :, grp * 32:(grp + 1) * 32]
                            .rearrange("p j (a b) -> p j a b", a=4),
                            ztp[:].rearrange("p a j b -> p j a b"))

                if hf == 0:
                    _frontend(1)
                c0 = hf * MCH
                csl = slice(c0, c0 + MCH)
                # xc planes for this half
                xrh2 = et.tile([128, 2, MCH], F32, tag="xrh2",
                               name=f"xrh2{hf}", bufs=1)
                xih2 = et.tile([128, 2, MCH], F32, tag="xih2",
                               name=f"xih2{hf}", bufs=1)
                for dd in range(2):
                    nc.vector.scalar_tensor_tensor(
                        xrh2[:, dd, :], zpl[0][dd][:, csl],
                        pcv[:, dd:dd + 1],
                        xsep[:, dd, csl], ALU.mult, ALU.add)
                    nc.vector.tensor_scalar_mul(
                        xih2[:, dd, :], zpl[1][dd][:, csl],
                        pcv[:, 2 + dd:3 + dd])

                # range reduction, Sin/Exp and magnitude products all
                # fused across both coordinate dims (halves op counts)
                msk = et.tile([128, 2, MCH], F32, tag="emsk", bufs=1)
                u2t = et.tile([128, 2, MCH], F32, tag="eu2", bufs=1)
                v2t = et.tile([128, 2, MCH], F32, tag="ev2", bufs=1)
                nc.vector.tensor_scalar(msk[:], xrh2[:], 0.5, None,
                                        ALU.is_ge)
                nc.vector.tensor_sub(u2t[:], xrh2[:], msk[:])
                nc.vector.tensor_scalar(msk[:], u2t[:], 0.25, None,
                                        ALU.add)
                nc.vector.tensor_scalar(v2t[:], msk[:], 0.5, None,
                                        ALU.is_ge)
                nc.vector.tensor_sub(v2t[:], msk[:], v2t[:])
                trs = et.tile([128, 2, MCH], F32, tag="etrs",
                              name=f"etrs{hf}", bufs=1)
                trc = et.tile([128, 2, MCH], F32, tag="etrc",
                              name=f"etrc{hf}", bufs=1)
                nc.scalar.activation(trs[:], u2t[:], ACT.Sin,
                                     scale=cc(C_TPI))
                nc.scalar.activation(trc[:], v2t[:], ACT.Sin,
                                     scale=cc(C_TPI))
                mdp = et.tile([128, 2, MCH], F32, tag="emdp",
                              name=f"emdp{hf}", bufs=1)
                mdm = et.tile([128, 2, MCH], F32, tag="emdm",
                              name=f"emdm{hf}", bufs=1)
                nc.scalar.activation(mdp[:], xih2[:], ACT.Exp,
                                     scale=cc(C_MTPI))
                nc.scalar.activation(mdm[:], xih2[:], ACT.Exp,
                                     scale=cc(C_TPI))
                frp = et.tile([128, 2, MCH], F32, tag="efrp",
                              name=f"efrp{hf}", bufs=1)
                fip = et.tile([128, 2, MCH], F32, tag="efip",
                              name=f"efip{hf}", bufs=1)
                frm = et.tile([128, 2, MCH], F32, tag="efrm",
                              name=f"efrm{hf}", bufs=1)
                fim = et.tile([128, 2, MCH], F32, tag="efim",
                              name=f"efim{hf}", bufs=1)
                nc.vector.tensor_mul(frp[:], mdp[:], trc[:])
                nc.vector.tensor_mul(fip[:], mdp[:], trs[:])
                nc.vector.tensor_mul(frm[:], mdm[:], trc[:])
                nc.vector.tensor_mul(fim[:], mdm[:], trs[:])
                names = {}
                for d2 in range(2):
                    names[(d2, 1)] = (frp[:, d2], fip[:, d2], 1)
                    names[(d2, -1)] = (frm[:, d2], fim[:, d2], -1)
                cols = []
                for (gx, gy) in gpat:
                    if gx != 0 and gy == 0:
                        cols.append(names[(0, gx)])
                    elif gx == 0 and gy != 0:
                        cols.append(names[(1, gy)])
                    else:
                        xr_, xi_, sx = names[(0, gx)]
                        yr_, yi_, sy = names[(1, gy)]
                        pre = et.tile([128, MCH], F32, tag=f"pr{gx}{gy}",
                                      name=f"pr{gx}{gy}{hf}", bufs=1)
                        pim = et.tile([128, MCH], F32, tag=f"pi{gx}{gy}",
                                      name=f"pi{gx}{gy}{hf}", bufs=1)
                        t1_ = et.tile([128, MCH], F32, tag="ept1", bufs=1)
                        t2_ = et.tile([128, MCH], F32, tag="ept2", bufs=1)
                        nc.gpsimd.tensor_mul(t1_[:], xr_, yr_)
                        nc.vector.tensor_mul(t2_[:], xi_, yi_)
                        nc.vector.tensor_tensor(
                            pre[:], t1_[:], t2_[:],
                            ALU.subtract if sx * sy > 0 else ALU.add)
                        nc.gpsimd.tensor_mul(t1_[:], xi_, yr_)
                        nc.vector.tensor_mul(t2_[:], xr_, yi_)
                        if sx > 0 and sy > 0:
                            nc.vector.tensor_add(pim[:], t1_[:], t2_[:])
                            isn = 1
                        elif sx < 0 and sy < 0:
                            nc.vector.tensor_add(pim[:], t1_[:], t2_[:])
                            isn = -1
                        elif sx > 0:
                            nc.vector.tensor_sub(pim[:], t1_[:], t2_[:])
                            isn = 1
                        else:
                            nc.vector.tensor_sub(pim[:], t2_[:], t1_[:])
                            isn = 1
                        cols.append((pre[:], pim[:], isn))

                # A-build (fused step-0 of the LU: col0 of E is all-ones)
                Arr = et.tile([128, BSH, 7, 7], F32, tag="Ar")
                Aii = et.tile([128, BSH, 7, 7], F32, tag="Ai")
                for j, (re, im, isn) in enumerate(cols):
                    rev = re.rearrange("p (t n) -> p t n", n=8)
                    imv = im.rearrange("p (t n) -> p t n", n=8)
                    nc.gpsimd.tensor_sub(
                        Arr[:, :, :, j], rev[:, :, 1:8],
                        rev[:, :, 0:1].broadcast_to((128, BSH, 7)))
                    if isn > 0:
                        nc.gpsimd.tensor_sub(
                            Aii[:, :, :, j], imv[:, :, 1:8],
                            imv[:, :, 0:1].broadcast_to((128, BSH, 7)))
                    else:
                        nc.gpsimd.tensor_sub(
                            Aii[:, :, :, j],
                            imv[:, :, 0:1].broadcast_to((128, BSH, 7)),
                            imv[:, :, 1:8])
                SPL = 23
                for k in range(6):
                    r = 6 - k
                    pr = Arr[:, :, k, k]
                    pi = Aii[:, :, k, k]
                    # raw columns expanded on ACT (the only strided-col
                    # read); row' = conj(P)/|P|^2 * row reads PACKED rows.
                    lrx = lt.tile([128, BSH, 6, 6], BF16, tag="lrx", bufs=1)
                    lix = lt.tile([128, BSH, 6, 6], BF16, tag="lix", bufs=1)
                    nc.scalar.copy(
                        lrx[:, :, 0:r, 0:r],
                        Arr[:, :, k + 1:7, k:k + 1].broadcast_to(
                            (128, BSH, r, r)))
                    nc.scalar.copy(
                        lix[:, :, 0:r, 0:r],
                        Aii[:, :, k + 1:7, k:k + 1].broadcast_to(
                            (128, BSH, r, r)))
                    t1 = st.tile([128, BSH], F32, tag="lt1", bufs=1)
                    t2 = st.tile([128, BSH], F32, tag="lt2", bufs=1)
                    nc.vector.tensor_mul(t1[:], pr, pr)
                    nc.vector.tensor_mul(t2[:], pi, pi)
                    nc.vector.tensor_add(t1[:], t1[:], t2[:])
                    rinv = st.tile([128, BSH], F32, tag="lrinv", bufs=1)
                    nc.vector.reciprocal(rinv[:], t1[:])
                    asc = st.tile([128, BSH], F32, tag="lasc", bufs=1)
                    bsc = st.tile([128, BSH], F32, tag="lbsc", bufs=1)
                    nc.vector.tensor_mul(asc[:], pr, rinv[:])
                    nc.vector.tensor_mul(bsc[:], pi, rinv[:])
                    rowr = Arr[:, :, k, k + 1:7]
                    rowi = Aii[:, :, k, k + 1:7]
                    abc = asc[:].unsqueeze(2).broadcast_to((128, BSH, r))
                    bbc = bsc[:].unsqueeze(2).broadcast_to((128, BSH, r))
                    m1 = st.tile([128, BSH, 6], F32, tag="lu1", bufs=1)
                    m2 = st.tile([128, BSH, 6], F32, tag="lu2", bufs=1)
                    rre = st.tile([128, BSH, 6], BF16, tag="llre", bufs=1)
                    rim = st.tile([128, BSH, 6], BF16, tag="llim", bufs=1)
                    m1v, m2v = m1[:, :, 0:r], m2[:, :, 0:r]
                    rrev, rimv = rre[:, :, 0:r], rim[:, :, 0:r]
                    nc.vector.tensor_mul(m1v, rowr, abc)
                    nc.vector.tensor_mul(m2v, rowi, bbc)
                    nc.vector.tensor_add(rrev, m1v, m2v)
                    nc.vector.tensor_mul(m1v, rowi, abc)
                    nc.vector.tensor_mul(m2v, rowr, bbc)
                    nc.vector.tensor_sub(rimv, m1v, m2v)
                    w1 = lt.tile([128, BSH, 6, 6], BF16, tag="lw1", bufs=1)
                    w2 = lt.tile([128, BSH, 6, 6], BF16, tag="lw2", bufs=1)
                    w3 = lt.tile([128, BSH, 6, 6], BF16, tag="lw3", bufs=1)
                    w4 = lt.tile([128, BSH, 6, 6], BF16, tag="lw4", bufs=1)
                    lreb = lrx[:, :, 0:r, 0:r]
                    limb = lix[:, :, 0:r, 0:r]
                    rre_bc = rrev.unsqueeze(2).broadcast_to(
                        (128, BSH, r, r))
                    rim_bc = rimv.unsqueeze(2).broadcast_to(
                        (128, BSH, r, r))
                    w1v = w1[:, :, 0:r, 0:r]
                    w2v = w2[:, :, 0:r, 0:r]
                    w3v = w3[:, :, 0:r, 0:r]
                    w4v = w4[:, :, 0:r, 0:r]
                    nc.vector.tensor_mul(w1v, lreb, rre_bc)
                    nc.vector.tensor_mul(w3v, lreb, rim_bc)
                    nc.vector.tensor_mul(w2v, limb, rim_bc)
                    nc.vector.tensor_sub(w1v, w1v, w2v)
                    nc.vector.tensor_sub(Arr[:, :, k + 1:7, k + 1:7],
                                         Arr[:, :, k + 1:7, k + 1:7], w1v)
                    nc.vector.tensor_mul(w4v, limb, rre_bc)
                    nc.vector.tensor_add(w3v, w3v, w4v)
                    nc.vector.tensor_sub(Aii[:, :, k + 1:7, k + 1:7],
                                         Aii[:, :, k + 1:7, k + 1:7], w3v)
                for k in range(7):
                    nc.vector.tensor_copy(prs[:, hf, :, k], Arr[:, :, k, k])
                    nc.vector.tensor_copy(pis[:, hf, :, k], Aii[:, :, k, k])

            # jastrow |d| -> min-image -> jv chain: ACT is idle during
            # the LU tail; only 2 DVE muls.  jv = A*(1-2*A*adm)^2 with
            # A = adm^2, via Square(-2C+1) fusing u and u^2.
            ja1 = jt.tile([128, NW, 240], BF16, tag="ja1")
            jA = jt.tile([128, NW, 240], BF16, tag="jA")
            nc.scalar.activation(ja1[:], jd[:], ACT.Abs)
            nc.scalar.activation(ja1[:], ja1[:], ACT.Abs,
                                 bias=cc(C_MHALF))
            nc.scalar.activation(jA[:], ja1[:], ACT.Square,
                                 scale=cc(C_MONE), bias=cc(C_HALF))
            nc.scalar.activation(ja1[:], ja1[:], ACT.Identity,
                                 scale=cc(C_MONE), bias=cc(C_HALF))
            nc.vector.tensor_mul(jd[:], jA[:], ja1[:])
            nc.scalar.activation(jd[:], jd[:], ACT.Square,
                                 scale=cc(C_MTWO), bias=cc(C_ONE))
            nc.vector.tensor_mul(jd[:], jA[:], jd[:])
            jdv = jd[:].rearrange("p b (q t) -> p b q t", t=2)
            nc.gpsimd.tensor_add(js[:], jdv[:, :, :, 0], jdv[:, :, :, 1])
            # jastrow sqrt + polynomial on tail-idle ACT/Pool
            s = js[:]
            tv = ja1[:, :, 0:120]
            p1 = ja1[:, :, 120:240]
            p2 = jA[:, :, 0:120]
            nc.scalar.activation(tv, s, ACT.Sqrt)
            nc.scalar.activation(p1, s, ACT.Identity,
                                 bias=cc(C_JP + 2), scale=cc(C_JP + 4))
            nc.gpsimd.tensor_tensor(p1, p1, s, ALU.mult)
            nc.scalar.activation(p1, p1, ACT.Identity,
                                 bias=cc(C_JP + 0))
            nc.scalar.activation(p2, s, ACT.Identity,
                                 bias=cc(C_JP + 3), scale=cc(C_JP + 5))
            nc.gpsimd.tensor_tensor(p2, p2, s, ALU.mult)
            nc.scalar.activation(p2, p2, ACT.Identity,
                                 bias=cc(C_JP + 1))
            nc.gpsimd.tensor_tensor(p1, p1, tv, ALU.mult)
            nc.gpsimd.tensor_tensor(p2, p2, s, ALU.mult)
            nc.vector.tensor_add(p1, p1, p2)
            nc.vector.tensor_reduce(jas[:], p1, axis=AXL.X, op=ALU.add)
            if debug:
                nc.sync.dma_start(dbg_jas[:], jas[:])

            # ===== batched logdet from the 2*BSH*7 saved pivots =====
            # |det|^2 = prod |p_k|^2 ; arg(det) = sum atan2(pi_k, pr_k)
            # wrapped to (-pi, pi].
            pv = prs[:].rearrange("p h t k -> p (h t k)")
            iv = pis[:].rearrange("p h t k -> p (h t k)")
            NP = 2 * BSH * 7
            den = lt.tile([128, NP], F32, tag="fden", bufs=1)
            tmp = lt.tile([128, NP], F32, tag="ftmp", bufs=1)
            nc.gpsimd.tensor_mul(den[:], pv, pv)
            nc.gpsimd.tensor_mul(tmp[:], iv, iv)
            nc.gpsimd.tensor_add(den[:], den[:], tmp[:])
            lnd = lt.tile([128, NP], F32, tag="ftmp", bufs=1)
            nc.scalar.activation(lnd[:], den[:], ACT.Ln)
            rp = lt.tile([128, NP], F32, tag="frp", bufs=1)
            nc.vector.reciprocal(rp[:], pv)
            nc.vector.tensor_mul(rp[:], iv, rp[:])
            at = lt.tile([128, NP], F32, tag="fden", bufs=1)
            nc.scalar.activation(at[:], rp[:], ACT.Arctan)
            m1 = lt.tile([128, NP], F32, tag="frp", bufs=1)
            m2 = lt.tile([128, NP], F32, tag="fm2", bufs=1)
            nc.vector.tensor_scalar(m1[:], pv, 0.0, None, ALU.is_lt)
            nc.vector.tensor_scalar(m2[:], iv, 0.0, None, ALU.is_ge)
            nc.vector.scalar_tensor_tensor(m2[:], m2[:], 2.0, m1[:],
                                           ALU.mult, ALU.mult)
            nc.vector.tensor_sub(m2[:], m2[:], m1[:])
            nc.vector.scalar_tensor_tensor(at[:], m2[:], PI, at[:],
                                           ALU.mult, ALU.add)
            labs = st.tile([128, 2, BSH], F32, tag="flabs")
            args = st.tile([128, 2, BSH], F32, tag="fargs")
            nc.vector.tensor_reduce(
                labs[:], lnd[:].rearrange("p (h t k) -> p h t k",
                                          h=2, k=7),
                axis=AXL.X, op=ALU.add)
            nc.vector.tensor_reduce(
                args[:], at[:].rearrange("p (h t k) -> p h t k",
                                         h=2, k=7),
                axis=AXL.X, op=ALU.add)
            wm = st.tile([128, 2, BSH], F32, tag="fwm")
            for _ in range(3):
                nc.vector.tensor_scalar(wm[:], args[:], PI, None, ALU.is_ge)
                nc.vector.scalar_tensor_tensor(args[:], wm[:], -2 * PI,
                                               args[:], ALU.mult, ALU.add)
                nc.vector.tensor_scalar(wm[:], args[:], -PI, None,
                                        ALU.is_lt)
                nc.vector.scalar_tensor_tensor(args[:], wm[:], 2 * PI,
                                               args[:], ALU.mult, ALU.add)
            for hf in range(2):
                labv = labs[:, hf].rearrange("p (tw s) -> p tw s", s=2)
                argv = args[:, hf].rearrange("p (tw s) -> p tw s", s=2)
                wsl = slice(hf * (NW // 2), (hf + 1) * (NW // 2))
                lsum = st.tile([128, NW // 2], F32, tag="olsum")
                nc.vector.tensor_add(lsum[:], labv[:, :, 0], labv[:, :, 1])
                nc.vector.scalar_tensor_tensor(
                    outri[:, wsl, 0], lsum[:], 0.5, jas[:, wsl],
                    ALU.mult, ALU.add)
                nc.vector.tensor_add(outri[:, wsl, 1], argv[:, :, 0],
                                     argv[:, :, 1])

            ov = out_d[:].rearrange("(p tw) r -> p tw r", p=128)
            nc.sync.dma_start(ov[:], outri[:])
            _lcm.__exit__(None, None, None)
    _legalize_waits(nc)
    if trace_sim:
        return nc, _tc_holder['tc']
    return nc


def _legalize_waits(nc):
    """This walrus build allows only ONE sync wait per instruction;
    Tile emits several. Split extras onto EventSemaphore nops."""
    n = 0
    for fn in nc.m.functions:
        for b in fn.blocks:
            out = []
            for ins in b.instructions:
                si = ins.sync_info
                if si is not None and si.on_wait and len(si.on_wait) > 1:
                    waits = list(si.on_wait)
                    for i, w in enumerate(waits[:-1]):
                        out.append(mybir.InstEventSemaphore(
                            name=f"WSPLIT{n}-{ins.name}",
                            engine=ins.engine,
                            sync_info=mybir.SyncInfo(on_wait=[w],
                                                     on_update=[]),
                            ins=[], outs=[], debug=ins.debug))
                        n += 1
                    ins.sync_info = mybir.SyncInfo(
                        on_wait=[waits[-1]],
                        on_update=list(si.on_update or []))
                out.append(ins)
            b.instructions = out
    return n


_CACHE = {}


def make_in_maps(inputs):
    cons, wv, ident, pcv = _host_precompute(inputs)
    kc = np.concatenate([cons, wv, ident, pcv], axis=1)
    x = np.ascontiguousarray(np.asarray(inputs["x"], np.float32))
    return [{
        "x": x[c * BLOC:(c + 1) * BLOC], "kc": kc,
    } for c in range(NCORES)]


def kernel(**inputs):
    if "nc" not in _CACHE:
        _CACHE["nc"] = build()
    nc = _CACHE["nc"]
    in_maps = make_in_maps(inputs)
    res = run_bass_kernel_spmd(nc, in_maps, core_ids=list(range(NCORES)))
    outs = [res.results[c]["out"] for c in range(NCORES)]
    full = np.concatenate(outs, axis=0)
    return (full[:, 0] + 1j * full[:, 1]).astype(np.complex64)



# revision 43
# speedup vs baseline: 1.0098x; 1.0098x over previous
"""Trainium2 Bass kernel for nn_Ansatz_44573170598544.

QMC ansatz: per-walker Jastrow + 2-qubit PQC backflow + two 8x8 complex
Slater log-determinants. Pure data parallel: 32768 walkers over 8 cores.

Host-precomputed transforms (validated vs reference in numpy):
  * PQC z = f^T Q f, f = kron of (cos,sin) of 4 half-angles;
    Q = V diag(lam) V^T -> z = sum_i lam_i (V^T f)_i^2: TensorE
    block-diag matmuls in feature-major layout via PE transposes.
  * Slater E[n,m] = exp(i r_n.k_m); kvec 0 is (0,0) -> column 0 all-ones
    -> LU step 0 is a row subtraction; then batched no-pivot LU on 7x7
    via stride-0 broadcast APs (8192 matrices/core).
  * logdet = 0.5*Ln(|det|^2) + i*atan2(Im,Re) per spin (matches
    log(sign)+logabs of slogdet; no branch-cut wrapping).

Layouts (per core, 4096 walkers):
  particle-major planes [128, 512]: particle m = part*512 + col.
  matrix (w,s) -> part p = (w*16+8*s)//512, chunk tc = ((w*16+8*s)%512)//8
    i.e. w = p*32 + tc//2, s = tc%2; its 8 particles are columns
    tc*8..tc*8+7 of partition p.  Walker w -> (p, tw=tc//2).
  Jastrow & output use the same walker mapping (p, tw).

Scheduling notes (engines have in-order queues; emission order matters):
  * per half: both q PE-feed groups, then that half's phasor/LU --
    LU0 overlaps hf1's PE/ACT feed (measured 18us faster than emitting
    all four feed groups before any phasor work).
  * jastrow differences/abs-chain fill the early DVE window; sqrt +
    jastrow polynomial run on the tail-idle ACT (Identity scale/bias
    passes) + Pool (tensor muls).
  * lambda-contraction matmuls use bf16 gsq weights (fp32 weights cost
    2 half-rate PE passes each).
  * LU rank-1 updates: lre/lim expanded to packed bf16 [t,r,r] tiles by
    ACT copies so the DVE products run in the 2x (2-byte) mode; the A
    accumulation stays f32.  Pivots stay on the A diagonal (never
    overwritten); logdet = sum Ln|p_k|^2 + i*sum atan2 over the 7 pivots,
    batched wide at the end, phase wrapped to (-pi, pi].
  * output written interleaved [128,32,2] -> one contiguous DMA (the
    strided re/im pair DMA cost ~70us in 4-byte descriptors).
"""
import sys
import numpy as np

sys.path.insert(0, "/opt/trn_rl_repo")

from concourse import bass, mybir, tile  # noqa: E402
from concourse.bass_utils import run_bass_kernel_spmd  # noqa: E402

F32 = mybir.dt.float32
BF16 = mybir.dt.bfloat16
ALU = mybir.AluOpType
ACT = mybir.ActivationFunctionType
AXL = mybir.AxisListType

NCORES = 8
B = 32768
BLOC = B // NCORES          # 4096 walkers/core
NW = BLOC // 128            # 32 walkers per partition
M = BLOC * 16               # 65536 particles/core
MC = M // 128               # 512 particle columns
MCH = MC // 2               # pqc/LU half: 256 cols = 32 mats/part
BSH = 32                    # matrices per partition per half
JCH = 2
JB = NW // JCH              # 16 walkers per jastrow chunk
PIH = float(np.pi / 2)
PI = float(np.pi)

C_JP = 0
C_SC = [6, 18]
C_BC = [10, 22]
C_BSN = [14, 26]
C_KX = 32
C_KY = 39
C_MHALF = 46
C_PIHC = 47
C_MONE = 48
C_TPI = 49
C_MTPI = 50
C_HALF = 51
C_MTWO = 52
C_ONE = 53
NCONS = 54
WQ = 144                    # per-q weights: W_both 128 (rank-8, j-fused), RL_both 16
NWV = 2 * WQ


def _host_precompute(inputs):
    def rz(t):
        e = np.exp(-0.5j * t)
        return np.diag([e, np.conj(e)])

    def ry(t):
        c, s = np.cos(0.5 * t), np.sin(0.5 * t)
        return np.array([[c, -s], [s, c]], complex)

    def euler(p):
        return rz(p[2]) @ ry(p[1]) @ rz(p[0])

    def entangler(t):
        I4 = np.eye(4, dtype=complex)
        dzz = np.array([1., -1., -1., 1.])
        XX = np.array([[0, 0, 0, 1], [0, 0, 1, 0], [0, 1, 0, 0],
                       [1, 0, 0, 0]], complex)
        YY = np.array([[0, 0, 0, -1], [0, 0, 1, 0], [0, 1, 0, 0],
                       [-1, 0, 0, 0]], complex)
        rzz = lambda a: np.diag(np.exp(-0.5j * a * dzz))
        rxx = np.cos(0.5 * t[1]) * I4 - 1j * np.sin(0.5 * t[1]) * XX
        ryy = np.cos(0.5 * t[2]) * I4 - 1j * np.sin(0.5 * t[2]) * YY
        return rzz(t[3]) @ ryy @ rxx @ rzz(t[0])

    cons = np.zeros((128, NCONS), np.float32)
    wv = np.zeros((128, NWV), np.float32)
    cons[:, C_JP:C_JP + 6] = np.asarray(inputs["jastrow_param"], np.float64)
    X = np.array([[0, 1], [1, 0]], complex)
    I2 = np.eye(2, dtype=complex)
    for q in range(2):
        sq = np.asarray(inputs["param_single_qubit"][q], np.float64)
        tq = np.asarray(inputs["param_two_qubit"][q], np.float64)
        enc = np.asarray(inputs["param_encoding"][q], np.float64)
        enc_b = np.asarray(inputs["param_encoding_bias"][q], np.float64)
        U1 = (np.kron(euler(sq[0, 2]), euler(sq[0, 3])) @ entangler(tq[0])
              @ np.kron(euler(sq[0, 0]), euler(sq[0, 1])))
        U2 = (np.kron(euler(sq[1, 2]), euler(sq[1, 3])) @ entangler(tq[1])
              @ np.kron(euler(sq[1, 0]), euler(sq[1, 1])))
        D1 = np.diag([1, -1j, -1j, -1]).astype(complex)
        M1 = U1 @ D1
        KS = [np.kron(I2, I2), np.kron(I2, -1j * X),
              np.kron(-1j * X, I2), np.kron(-1j * X, -1j * X)]
        T = np.zeros((4, 16), complex)
        for i1 in range(2):
            for i2 in range(2):
                e12 = np.zeros(4)
                e12[2 * i1 + i2] = 1.0
                base = M1 @ e12
                for i3 in range(2):
                    for i4 in range(2):
                        T[:, 8 * i1 + 4 * i2 + 2 * i3 + i4] = \
                            U2 @ KS[2 * i3 + i4] @ base
        Z0 = np.diag([1., 1., -1., -1.]).astype(complex)
        Z1 = np.diag([1., -1., 1., -1.]).astype(complex)
        # Q has exact rank 8; keep the 8 nonzero eigenpairs and fuse the
        # two observables (j) into one weight block: out part = (j,a2,c).
        Wb = np.zeros((128, 128), np.float32)
        RLb = np.zeros((128, 16), np.float32)
        for j, Z in enumerate((Z0, Z1)):
            Q = np.real(T.conj().T @ Z @ T)
            Q = 0.5 * (Q + Q.T)
            lam, V = np.linalg.eigh(Q)
            idx8 = np.argsort(np.abs(lam))[-8:]
            V8 = V[:, idx8].astype(np.float32)
            lam8 = lam[idx8].astype(np.float32)
            for c in range(8):
                rows = np.arange(16) * 8 + c
                colsb = j * 64 + np.arange(8) * 8 + c
                Wb[np.ix_(rows, colsb)] = V8
                RLb[colsb, j * 8 + c] = lam8
        wv[:, q * WQ: q * WQ + 128] = Wb
        wv[:, q * WQ + 128: q * WQ + 144] = RLb
        scale = np.array([enc[0, 0], enc[0, 1], enc[1, 0], enc[1, 1]]) * np.pi
        bias = 0.5 * np.array([enc_b[0, 0], enc_b[0, 1],
                               enc_b[1, 0], enc_b[1, 1]])
        cons[:, C_SC[q]:C_SC[q] + 4] = scale
        cons[:, C_BC[q]:C_BC[q] + 4] = bias + PIH
        cons[:, C_BSN[q]:C_BSN[q] + 4] = bias
    kv = np.asarray(inputs["kvecs"], np.float64)
    assert abs(kv[0]).max() < 1e-6, "kernel assumes kvecs[0] == 0"
    cons[:, C_KX:C_KX + 7] = kv[1:8, 0]
    cons[:, C_KY:C_KY + 7] = kv[1:8, 1]
    cons[:, C_MHALF] = -0.5
    cons[:, C_PIHC] = PIH
    cons[:, C_MONE] = -1.0
    cons[:, C_TPI] = 2 * PI
    cons[:, C_MTPI] = -2 * PI
    cons[:, C_HALF] = 0.5
    cons[:, C_MTWO] = -2.0
    cons[:, C_ONE] = 1.0
    pc = np.asarray(inputs["param_classical"], np.float64)
    pcv = np.zeros((128, 4), np.float32)
    pcv[:, 0:2] = pc[0]
    pcv[:, 2:4] = pc[1]
    ident = np.eye(128, dtype=np.float32)
    return cons, wv, ident, pcv


def build(gpat=None, debug=False, loop_n=0, trace_sim=False):
    if gpat is None:
        gpat = ((-1, 0), (0, -1), (0, 1), (1, 0),
                (-1, -1), (-1, 1), (1, -1))
    nc = bass.Bass()
    x_d = nc.declare_dram_parameter("x", [BLOC, 32], F32, isOutput=False)
    NKC = NCONS + NWV + 128 + 4
    kc_d = nc.declare_dram_parameter("kc", [128, NKC], F32, isOutput=False)
    out_d = nc.declare_dram_parameter("out", [BLOC, 2], F32, isOutput=True)
    if debug:
        dbg_jas = nc.declare_dram_parameter("dbg_jas", [128, NW], F32,
                                            isOutput=True)
        dbg_z = nc.declare_dram_parameter("dbg_z", [128, 4, MC], F32,
                                          isOutput=True)
        dbg_E = nc.declare_dram_parameter("dbg_E", [128, 2, BSH, 8, 7], F32,
                                          isOutput=True)
        dbg_det = nc.declare_dram_parameter("dbg_det", [128, 4, BSH], F32,
                                            isOutput=True)

    xflat = x_d[:].rearrange("b c -> (b c)")

    _tc_holder = {}
    with tile.TileContext(nc, trace_sim=trace_sim) as tc:
        _tc_holder['tc'] = tc
        with (
            tc.tile_pool(name="const", bufs=1) as cpool,
            tc.tile_pool(name="pers", bufs=1) as pers,
            tc.tile_pool(name="jt", bufs=1) as jt,
            tc.tile_pool(name="pt", bufs=1) as pt,
            tc.tile_pool(name="gt", bufs=4) as gt,
            tc.tile_pool(name="et", bufs=2) as et,
            tc.tile_pool(name="lt", bufs=2) as lt,
            tc.tile_pool(name="st", bufs=2) as st,
            tc.tile_pool(name="ps_t", bufs=3, space="PSUM") as ps_t,
            tc.tile_pool(name="ps_g", bufs=3, space="PSUM") as ps_g,
            tc.tile_pool(name="ps_w", bufs=1, space="PSUM") as ps_w,
        ):
            kc = cpool.tile([128, NKC], F32, tag="kc")
            nc.sync.dma_start(kc[:], kc_d[:])
            cons = kc[:, 0:NCONS]
            wvt = kc[:, NCONS:NCONS + NWV]
            ident = kc[:, NCONS + NWV:NCONS + NWV + 128]
            pcv = kc[:, NCONS + NWV + 128:NCONS + NWV + 132]

            wvb = cpool.tile([128, NWV], BF16, tag="wvb")
            nc.scalar.copy(wvb[:], kc[:, NCONS:NCONS + NWV])
            identb = cpool.tile([128, 128], BF16, tag="identb")
            nc.scalar.copy(identb[:], ident)

            def cc(i):
                return cons[:, i:i + 1]

            # =============== Jastrow (walker (p,tw) mapping) ===========
            xin = pers.tile([128, NW, 32], F32, tag="xin")
            xdv = xflat.rearrange("(p tw c) -> p tw c", p=128, tw=NW, c=32)
            # split the input DMA by walker halves: half-0 compute
            # (jastrow chunk 0, hf0 trig) starts after the first half lands
            nc.sync.dma_start(xin[:, 0:NW // 2, :], xdv[:, 0:NW // 2, :])
            nc.sync.dma_start(xin[:, NW // 2:NW, :], xdv[:, NW // 2:NW, :])
            xall = xin[:].rearrange("p tw c -> p (tw c)").rearrange(
                "p (cc d) -> p cc d", d=2)
            xsep = pers.tile([128, 2, MC], F32, tag="xsep")
            for hx in range(2):
                hs = slice(hx * MCH, (hx + 1) * MCH)
                nc.vector.tensor_copy(xsep[:, 0, hs], xall[:, hs, 0])
                nc.vector.tensor_copy(xsep[:, 1, hs], xall[:, hs, 1])
            # ScalarE warm-ups: observe each DMA queue once so no real ACT
            # instruction ever needs two semaphore waits (ISA limit is 1).
            wsc1 = cpool.tile([128, 1], F32, tag="wsc1")
            wsc2 = cpool.tile([128, 1], F32, tag="wsc2")
            nc.scalar.activation(wsc1[:], kc[:, 0:1], ACT.Copy)
            nc.scalar.activation(wsc2[:], xin[:, 0, 0:1], ACT.Copy)
            wps = ps_w.tile([8, 8], F32, tag="wps")
            nc.tensor.transpose(wps[:], ident[0:8, 0:8], ident[0:8, 0:8])
            import contextlib
            _lcm = tc.For_i(0, loop_n, 1) if loop_n else \
                contextlib.nullcontext()
            _lcm.__enter__()
            # =============== PQC backflow -> zplm[q][:,j,:] ============
            zplm = [pers.tile([128, 2, MC], F32, tag=f"zplm{q}",
                              name=f"zplm{q}") for q in range(2)]
            zpl = [[zplm[q][:, j] for j in range(2)] for q in range(2)]


            # PQC frontend for all (hf,q) first: trig (ACT) + feature
            # products (DVE) so the PE transpose/matmul pipeline starts
            # immediately; jastrow then fills ACT/DVE/Pool gaps.
            fts = {}

            def _frontend(hf):
                c0 = hf * MCH
                for q in range(2):
                    trig = pt.tile([128, 8, MCH], BF16, tag="trig")
                    for j in range(4):
                        coord = xsep[:, j % 2, c0:c0 + MCH]
                        nc.scalar.activation(trig[:, 2 * j, :], coord,
                                             ACT.Sin, bias=cc(C_BC[q] + j),
                                             scale=cc(C_SC[q] + j))
                        nc.scalar.activation(trig[:, 2 * j + 1, :], coord,
                                             ACT.Sin, bias=cc(C_BSN[q] + j),
                                             scale=cc(C_SC[q] + j))
                    u = pt.tile([128, 2, 2, MCH], BF16, tag="u")
                    nc.vector.tensor_mul(
                        u[:],
                        trig[:, 0:2, :].unsqueeze(2).broadcast_to(
                            (128, 2, 2, MCH)),
                        trig[:, 2:4, :].unsqueeze(1).broadcast_to(
                            (128, 2, 2, MCH)))
                    v = pt.tile([128, 2, 2, MCH], BF16, tag="v")
                    nc.vector.tensor_mul(
                        v[:],
                        trig[:, 4:6, :].unsqueeze(2).broadcast_to(
                            (128, 2, 2, MCH)),
                        trig[:, 6:8, :].unsqueeze(1).broadcast_to(
                            (128, 2, 2, MCH)))
                    f = pt.tile([128, MCH // 8, 16, 8], BF16,
                                tag=f"f{hf}{q}")
                    fo = f[:].rearrange("p t (a b) c -> p a b t c", a=4)
                    nc.vector.tensor_mul(
                        fo,
                        u[:].rearrange("p a b (t c) -> p (a b) t c", c=8)
                            .unsqueeze(2).broadcast_to(
                                (128, 4, 4, MCH // 8, 8)),
                        v[:].rearrange("p a b (t c) -> p (a b) t c", c=8)
                            .unsqueeze(1).broadcast_to(
                                (128, 4, 4, MCH // 8, 8)))
                    fts[(hf, q)] = f

            _frontend(0)

            # ===== Jastrow front: fused pair differences only (DVE).
            # The |d|/min-image/poly chain runs post-feed on the
            # tail-idle ACT (see below).
            jas = pers.tile([128, NW], F32, tag="jas")
            js = pers.tile([128, NW, 120], BF16, tag="js")
            jd = jt.tile([128, NW, 240], BF16, tag="jd")
            off = 0
            for o in range(1, 16):
                Lg = 32 - 2 * o
                nc.vector.tensor_sub(jd[:, :, off:off + Lg],
                                     xin[:, :, 0:Lg],
                                     xin[:, :, 2 * o:32])
                off += Lg
            outri = pers.tile([128, NW, 2], F32, tag="outri")
            prs = pers.tile([128, 2, BSH, 7], F32, tag="prs")
            pis = pers.tile([128, 2, BSH, 7], F32, tag="pis")
            # PE feed for all four (hf,q) groups first — keeps the
            # in-order ACT queue free of phasor ops that would stall it.
            for hf in range(2):
                for q in range(2):
                    f = fts[(hf, q)]
                    for gl in range(8):
                        grp = hf * 8 + gl
                        ftp = ps_t.tile([128, 512], BF16, tag="ftp")
                        for gi in range(4):
                            ti = gl * 4 + gi
                            nc.tensor.transpose(
                                ftp[:, gi * 128:(gi + 1) * 128],
                                f[:, ti].rearrange("p a c -> p (a c)"),
                                identb[:])
                        ftr = gt.tile([128, 512], BF16, tag="ftr")
                        nc.scalar.copy(ftr[:], ftp[:])
                        gp = ps_g.tile([128, 512], F32, tag="gp")
                        ztp = ps_w.tile([128, 4, 2, 8], F32, tag="ztp")
                        nc.tensor.matmul(
                            gp[:],
                            wvb[:, q * WQ:q * WQ + 128],
                            ftr[:])
                        gsq = gt.tile([128, 512], BF16, tag="gsq")
                        nc.scalar.activation(gsq[:], gp[:], ACT.Square)
                        for gi in range(4):
                            nc.tensor.matmul(
                                ztp[:, gi, :, :],
                                gsq[:, gi * 128:(gi + 1) * 128],
                                wvb[:, q * WQ + 128:q * WQ + 144])
                        nc.scalar.copy(
                            zplm[q][:, :, grp * 32:(grp + 1) * 32]
                            .rearrange("p j (a b) -> p j a b", a=4),
                            ztp[:].rearrange("p a j b -> p j a b"))

                if hf == 0:
                    _frontend(1)
                c0 = hf * MCH
                csl = slice(c0, c0 + MCH)
                # xc planes for this half
                xrh2 = et.tile([128, 2, MCH], F32, tag="xrh2",
                               name=f"xrh2{hf}", bufs=1)
                xih2 = et.tile([128, 2, MCH], F32, tag="xih2",
                               name=f"xih2{hf}", bufs=1)
                for dd in range(2):
                    nc.vector.scalar_tensor_tensor(
                        xrh2[:, dd, :], zpl[0][dd][:, csl],
                        pcv[:, dd:dd + 1],
                        xsep[:, dd, csl], ALU.mult, ALU.add)
                    nc.vector.tensor_scalar_mul(
                        xih2[:, dd, :], zpl[1][dd][:, csl],
                        pcv[:, 2 + dd:3 + dd])

                # range reduction, Sin/Exp and magnitude products all
                # fused across both coordinate dims (halves op counts)
                msk = et.tile([128, 2, MCH], F32, tag="emsk", bufs=1)
                u2t = et.tile([128, 2, MCH], F32, tag="eu2", bufs=1)
                v2t = et.tile([128, 2, MCH], F32, tag="ev2", bufs=1)
                nc.vector.tensor_scalar(msk[:], xrh2[:], 0.5, None,
                                        ALU.is_ge)
                nc.vector.tensor_sub(u2t[:], xrh2[:], msk[:])
                nc.vector.tensor_scalar(msk[:], u2t[:], 0.25, None,
                                        ALU.add)
                nc.vector.tensor_scalar(v2t[:], msk[:], 0.5, None,
                                        ALU.is_ge)
                nc.vector.tensor_sub(v2t[:], msk[:], v2t[:])
                trs = et.tile([128, 2, MCH], F32, tag="etrs",
                              name=f"etrs{hf}", bufs=1)
                trc = et.tile([128, 2, MCH], F32, tag="etrc",
                              name=f"etrc{hf}", bufs=1)
                nc.scalar.activation(trs[:], u2t[:], ACT.Sin,
                                     scale=cc(C_TPI))
                nc.scalar.activation(trc[:], v2t[:], ACT.Sin,
                                     scale=cc(C_TPI))
                mdp = et.tile([128, 2, MCH], F32, tag="emdp",
                              name=f"emdp{hf}", bufs=1)
                mdm = et.tile([128, 2, MCH], F32, tag="emdm",
                              name=f"emdm{hf}", bufs=1)
                nc.scalar.activation(mdp[:], xih2[:], ACT.Exp,
                                     scale=cc(C_MTPI))
                nc.scalar.activation(mdm[:], xih2[:], ACT.Exp,
                                     scale=cc(C_TPI))
                frp = et.tile([128, 2, MCH], F32, tag="efrp",
                              name=f"efrp{hf}", bufs=1)
                fip = et.tile([128, 2, MCH], F32, tag="efip",
                              name=f"efip{hf}", bufs=1)
                frm = et.tile([128, 2, MCH], F32, tag="efrm",
                              name=f"efrm{hf}", bufs=1)
                fim = et.tile([128, 2, MCH], F32, tag="efim",
                              name=f"efim{hf}", bufs=1)
                nc.vector.tensor_mul(frp[:], mdp[:], trc[:])
                nc.vector.tensor_mul(fip[:], mdp[:], trs[:])
                nc.vector.tensor_mul(frm[:], mdm[:], trc[:])
                nc.vector.tensor_mul(fim[:], mdm[:], trs[:])
                names = {}
                for d2 in range(2):
                    names[(d2, 1)] = (frp[:, d2], fip[:, d2], 1)
                    names[(d2, -1)] = (frm[:, d2], fim[:, d2], -1)
                cols = []
                for (gx, gy) in gpat:
                    if gx != 0 and gy == 0:
                        cols.append(names[(0, gx)])
                    elif gx == 0 and gy != 0:
                        cols.append(names[(1, gy)])
                    else:
                        xr_, xi_, sx = names[(0, gx)]
                        yr_, yi_, sy = names[(1, gy)]
                        pre = et.tile([128, MCH], F32, tag=f"pr{gx}{gy}",
                                      name=f"pr{gx}{gy}{hf}", bufs=1)
                        pim = et.tile([128, MCH], F32, tag=f"pi{gx}{gy}",
                                      name=f"pi{gx}{gy}{hf}", bufs=1)
                        t1_ = et.tile([128, MCH], F32, tag="ept1", bufs=1)
                        t2_ = et.tile([128, MCH], F32, tag="ept2", bufs=1)
                        nc.gpsimd.tensor_mul(t1_[:], xr_, yr_)
                        nc.vector.tensor_mul(t2_[:], xi_, yi_)
                        nc.vector.tensor_tensor(
                            pre[:], t1_[:], t2_[:],
                            ALU.subtract if sx * sy > 0 else ALU.add)
                        nc.gpsimd.tensor_mul(t1_[:], xi_, yr_)
                        nc.vector.tensor_mul(t2_[:], xr_, yi_)
                        if sx > 0 and sy > 0:
                            nc.vector.tensor_add(pim[:], t1_[:], t2_[:])
                            isn = 1
                        elif sx < 0 and sy < 0:
                            nc.vector.tensor_add(pim[:], t1_[:], t2_[:])
                            isn = -1
                        elif sx > 0:
                            nc.vector.tensor_sub(pim[:], t1_[:], t2_[:])
                            isn = 1
                        else:
                            nc.vector.tensor_sub(pim[:], t2_[:], t1_[:])
                            isn = 1
                        cols.append((pre[:], pim[:], isn))

                # A-build (fused step-0 of the LU: col0 of E is all-ones)
                Arr = et.tile([128, BSH, 7, 7], F32, tag="Ar")
                Aii = et.tile([128, BSH, 7, 7], F32, tag="Ai")
                for j, (re, im, isn) in enumerate(cols):
                    rev = re.rearrange("p (t n) -> p t n", n=8)
                    imv = im.rearrange("p (t n) -> p t n", n=8)
                    # transposed build (S = A^T, det identical): row
                    # writes are packed on Pool instead of stride-7 cols
                    nc.gpsimd.tensor_sub(
                        Arr[:, :, j, :], rev[:, :, 1:8],
                        rev[:, :, 0:1].broadcast_to((128, BSH, 7)))
                    if isn > 0:
                        nc.gpsimd.tensor_sub(
                            Aii[:, :, j, :], imv[:, :, 1:8],
                            imv[:, :, 0:1].broadcast_to((128, BSH, 7)))
                    else:
                        nc.gpsimd.tensor_sub(
                            Aii[:, :, j, :],
                            imv[:, :, 0:1].broadcast_to((128, BSH, 7)),
                            imv[:, :, 1:8])
                SPL = 23
                for k in range(6):
                    r = 6 - k
                    pr = Arr[:, :, k, k]
                    pi = Aii[:, :, k, k]
                    # raw columns expanded on ACT (the only strided-col
                    # read); row' = conj(P)/|P|^2 * row reads PACKED rows.
                    lrx = lt.tile([128, BSH, 6, 6], BF16, tag="lrx", bufs=1)
                    lix = lt.tile([128, BSH, 6, 6], BF16, tag="lix", bufs=1)
                    nc.scalar.copy(
                        lrx[:, :, 0:r, 0:r],
                        Arr[:, :, k + 1:7, k:k + 1].broadcast_to(
                            (128, BSH, r, r)))
                    nc.scalar.copy(
                        lix[:, :, 0:r, 0:r],
                        Aii[:, :, k + 1:7, k:k + 1].broadcast_to(
                            (128, BSH, r, r)))
                    t1 = st.tile([128, BSH], F32, tag="lt1", bufs=1)
                    t2 = st.tile([128, BSH], F32, tag="lt2", bufs=1)
                    nc.vector.tensor_mul(t1[:], pr, pr)
                    nc.vector.tensor_mul(t2[:], pi, pi)
                    nc.vector.tensor_add(t1[:], t1[:], t2[:])
                    rinv = st.tile([128, BSH], F32, tag="lrinv", bufs=1)
                    nc.vector.reciprocal(rinv[:], t1[:])
                    asc = st.tile([128, BSH], F32, tag="lasc", bufs=1)
                    bsc = st.tile([128, BSH], F32, tag="lbsc", bufs=1)
                    nc.vector.tensor_mul(asc[:], pr, rinv[:])
                    nc.vector.tensor_mul(bsc[:], pi, rinv[:])
                    rowr = Arr[:, :, k, k + 1:7]
                    rowi = Aii[:, :, k, k + 1:7]
                    abc = asc[:].unsqueeze(2).broadcast_to((128, BSH, r))
                    bbc = bsc[:].unsqueeze(2).broadcast_to((128, BSH, r))
                    m1 = st.tile([128, BSH, 6], F32, tag="lu1", bufs=1)
                    m2 = st.tile([128, BSH, 6], F32, tag="lu2", bufs=1)
                    rre = st.tile([128, BSH, 6], BF16, tag="llre", bufs=1)
                    rim = st.tile([128, BSH, 6], BF16, tag="llim", bufs=1)
                    m1v, m2v = m1[:, :, 0:r], m2[:, :, 0:r]
                    rrev, rimv = rre[:, :, 0:r], rim[:, :, 0:r]
                    nc.vector.tensor_mul(m1v, rowr, abc)
                    nc.vector.tensor_mul(m2v, rowi, bbc)
                    nc.vector.tensor_add(rrev, m1v, m2v)
                    nc.vector.tensor_mul(m1v, rowi, abc)
                    nc.vector.tensor_mul(m2v, rowr, bbc)
                    nc.vector.tensor_sub(rimv, m1v, m2v)
                    w1 = lt.tile([128, BSH, 6, 6], BF16, tag="lw1", bufs=1)
                    w2 = lt.tile([128, BSH, 6, 6], BF16, tag="lw2", bufs=1)
                    w3 = lt.tile([128, BSH, 6, 6], BF16, tag="lw3", bufs=1)
                    w4 = lt.tile([128, BSH, 6, 6], BF16, tag="lw4", bufs=1)
                    lreb = lrx[:, :, 0:r, 0:r]
                    limb = lix[:, :, 0:r, 0:r]
                    rre_bc = rrev.unsqueeze(2).broadcast_to(
                        (128, BSH, r, r))
                    rim_bc = rimv.unsqueeze(2).broadcast_to(
                        (128, BSH, r, r))
                    w1v = w1[:, :, 0:r, 0:r]
                    w2v = w2[:, :, 0:r, 0:r]
                    w3v = w3[:, :, 0:r, 0:r]
                    w4v = w4[:, :, 0:r, 0:r]
                    nc.vector.tensor_mul(w1v, lreb, rre_bc)
                    nc.vector.tensor_mul(w3v, lreb, rim_bc)
                    nc.vector.tensor_mul(w2v, limb, rim_bc)
                    nc.vector.tensor_sub(w1v, w1v, w2v)
                    nc.vector.tensor_sub(Arr[:, :, k + 1:7, k + 1:7],
                                         Arr[:, :, k + 1:7, k + 1:7], w1v)
                    nc.vector.tensor_mul(w4v, limb, rre_bc)
                    nc.vector.tensor_add(w3v, w3v, w4v)
                    nc.vector.tensor_sub(Aii[:, :, k + 1:7, k + 1:7],
                                         Aii[:, :, k + 1:7, k + 1:7], w3v)
                for k in range(7):
                    nc.vector.tensor_copy(prs[:, hf, :, k], Arr[:, :, k, k])
                    nc.vector.tensor_copy(pis[:, hf, :, k], Aii[:, :, k, k])

            # jastrow |d| -> min-image -> jv chain: ACT is idle during
            # the LU tail; only 2 DVE muls.  jv = A*(1-2*A*adm)^2 with
            # A = adm^2, via Square(-2C+1) fusing u and u^2.
            ja1 = jt.tile([128, NW, 240], BF16, tag="ja1")
            jA = jt.tile([128, NW, 240], BF16, tag="jA")
            nc.scalar.activation(ja1[:], jd[:], ACT.Abs)
            nc.scalar.activation(ja1[:], ja1[:], ACT.Abs,
                                 bias=cc(C_MHALF))
            nc.scalar.activation(jA[:], ja1[:], ACT.Square,
                                 scale=cc(C_MONE), bias=cc(C_HALF))
            nc.scalar.activation(ja1[:], ja1[:], ACT.Identity,
                                 scale=cc(C_MONE), bias=cc(C_HALF))
            nc.vector.tensor_mul(jd[:], jA[:], ja1[:])
            nc.scalar.activation(jd[:], jd[:], ACT.Square,
                                 scale=cc(C_MTWO), bias=cc(C_ONE))
            nc.vector.tensor_mul(jd[:], jA[:], jd[:])
            jdv = jd[:].rearrange("p b (q t) -> p b q t", t=2)
            nc.gpsimd.tensor_add(js[:], jdv[:, :, :, 0], jdv[:, :, :, 1])
            # jastrow sqrt + polynomial on tail-idle ACT/Pool
            s = js[:]
            tv = ja1[:, :, 0:120]
            p1 = ja1[:, :, 120:240]
            p2 = jA[:, :, 0:120]
            nc.scalar.activation(tv, s, ACT.Sqrt)
            nc.scalar.activation(p1, s, ACT.Identity,
                                 bias=cc(C_JP + 2), scale=cc(C_JP + 4))
            nc.gpsimd.tensor_tensor(p1, p1, s, ALU.mult)
            nc.scalar.activation(p1, p1, ACT.Identity,
                                 bias=cc(C_JP + 0))
            nc.scalar.activation(p2, s, ACT.Identity,
                                 bias=cc(C_JP + 3), scale=cc(C_JP + 5))
            nc.gpsimd.tensor_tensor(p2, p2, s, ALU.mult)
            nc.scalar.activation(p2, p2, ACT.Identity,
                                 bias=cc(C_JP + 1))
            nc.gpsimd.tensor_tensor(p1, p1, tv, ALU.mult)
            nc.gpsimd.tensor_tensor(p2, p2, s, ALU.mult)
            nc.vector.tensor_add(p1, p1, p2)
            nc.vector.tensor_reduce(jas[:], p1, axis=AXL.X, op=ALU.add)
            if debug:
                nc.sync.dma_start(dbg_jas[:], jas[:])

            # ===== batched logdet from the 2*BSH*7 saved pivots =====
            # |det|^2 = prod |p_k|^2 ; arg(det) = sum atan2(pi_k, pr_k)
            # wrapped to (-pi, pi].
            pv = prs[:].rearrange("p h t k -> p (h t k)")
            iv = pis[:].rearrange("p h t k -> p (h t k)")
            NP = 2 * BSH * 7
            den = lt.tile([128, NP], F32, tag="fden", bufs=1)
            tmp = lt.tile([128, NP], F32, tag="ftmp", bufs=1)
            nc.gpsimd.tensor_mul(den[:], pv, pv)
            nc.gpsimd.tensor_mul(tmp[:], iv, iv)
            nc.gpsimd.tensor_add(den[:], den[:], tmp[:])
            lnd = lt.tile([128, NP], F32, tag="ftmp", bufs=1)
            nc.scalar.activation(lnd[:], den[:], ACT.Ln)
            rp = lt.tile([128, NP], F32, tag="frp", bufs=1)
            nc.vector.reciprocal(rp[:], pv)
            nc.vector.tensor_mul(rp[:], iv, rp[:])
            at = lt.tile([128, NP], F32, tag="fden", bufs=1)
            nc.scalar.activation(at[:], rp[:], ACT.Arctan)
            m1 = lt.tile([128, NP], F32, tag="frp", bufs=1)
            m2 = lt.tile([128, NP], F32, tag="fm2", bufs=1)
            nc.vector.tensor_scalar(m1[:], pv, 0.0, None, ALU.is_lt)
            nc.vector.tensor_scalar(m2[:], iv, 0.0, None, ALU.is_ge)
            nc.vector.scalar_tensor_tensor(m2[:], m2[:], 2.0, m1[:],
                                           ALU.mult, ALU.mult)
            nc.vector.tensor_sub(m2[:], m2[:], m1[:])
            nc.vector.scalar_tensor_tensor(at[:], m2[:], PI, at[:],
                                           ALU.mult, ALU.add)
            labs = st.tile([128, 2, BSH], F32, tag="flabs")
            args = st.tile([128, 2, BSH], F32, tag="fargs")
            nc.vector.tensor_reduce(
                labs[:], lnd[:].rearrange("p (h t k) -> p h t k",
                                          h=2, k=7),
                axis=AXL.X, op=ALU.add)
            nc.vector.tensor_reduce(
                args[:], at[:].rearrange("p (h t k) -> p h t k",
                                         h=2, k=7),
                axis=AXL.X, op=ALU.add)
            wm = st.tile([128, 2, BSH], F32, tag="fwm")
            for _ in range(3):
                nc.vector.tensor_scalar(wm[:], args[:], PI, None, ALU.is_ge)
                nc.vector.scalar_tensor_tensor(args[:], wm[:], -2 * PI,
                                               args[:], ALU.mult, ALU.add)
                nc.vector.tensor_scalar(wm[:], args[:], -PI, None,
                                        ALU.is_lt)
                nc.vector.scalar_tensor_tensor(args[:], wm[:], 2 * PI,
                                               args[:], ALU.mult, ALU.add)
            for hf in range(2):
                labv = labs[:, hf].rearrange("p (tw s) -> p tw s", s=2)
                argv = args[:, hf].rearrange("p (tw s) -> p tw s", s=2)
                wsl = slice(hf * (NW // 2), (hf + 1) * (NW // 2))
                lsum = st.tile([128, NW // 2], F32, tag="olsum")
                nc.vector.tensor_add(lsum[:], labv[:, :, 0], labv[:, :, 1])
                nc.vector.scalar_tensor_tensor(
                    outri[:, wsl, 0], lsum[:], 0.5, jas[:, wsl],
                    ALU.mult, ALU.add)
                nc.vector.tensor_add(outri[:, wsl, 1], argv[:, :, 0],
                                     argv[:, :, 1])

            ov = out_d[:].rearrange("(p tw) r -> p tw r", p=128)
            nc.sync.dma_start(ov[:], outri[:])
            _lcm.__exit__(None, None, None)
    _legalize_waits(nc)
    if trace_sim:
        return nc, _tc_holder['tc']
    return nc


def _legalize_waits(nc):
    """This walrus build allows only ONE sync wait per instruction;
    Tile emits several. Split extras onto EventSemaphore nops."""
    n = 0
    for fn in nc.m.functions:
        for b in fn.blocks:
            out = []
            for ins in b.instructions:
                si = ins.sync_info
                if si is not None and si.on_wait and len(si.on_wait) > 1:
                    waits = list(si.on_wait)
                    for i, w in enumerate(waits[:-1]):
                        out.append(mybir.InstEventSemaphore(
                            name=f"WSPLIT{n}-{ins.name}",
                            engine=ins.engine,
                            sync_info=mybir.SyncInfo(on_wait=[w],
                                                     on_update=[]),
                            ins=[], outs=[], debug=ins.debug))
                        n += 1
                    ins.sync_info = mybir.SyncInfo(
                        on_wait=[waits[-1]],
                        on_update=list(si.on_update or []))
                out.append(ins)
            b.instructions = out
    return n


_CACHE = {}


def make_in_maps(inputs):
    cons, wv, ident, pcv = _host_precompute(inputs)
    kc = np.concatenate([cons, wv, ident, pcv], axis=1)
    x = np.ascontiguousarray(np.asarray(inputs["x"], np.float32))
    return [{
        "x": x[c * BLOC:(c + 1) * BLOC], "kc": kc,
    } for c in range(NCORES)]


def kernel(**inputs):
    if "nc" not in _CACHE:
        _CACHE["nc"] = build()
    nc = _CACHE["nc"]
    in_maps = make_in_maps(inputs)
    res = run_bass_kernel_spmd(nc, in_maps, core_ids=list(range(NCORES)))
    outs = [res.results[c]["out"] for c in range(NCORES)]
    full = np.concatenate(outs, axis=0)
    return (full[:, 0] + 1j * full[:, 1]).astype(np.complex64)



# revision 46
# speedup vs baseline: 1.0222x; 1.0123x over previous
"""Trainium2 Bass kernel for nn_Ansatz_44573170598544.

QMC ansatz: per-walker Jastrow + 2-qubit PQC backflow + two 8x8 complex
Slater log-determinants. Pure data parallel: 32768 walkers over 8 cores.

Host-precomputed transforms (validated vs reference in numpy):
  * PQC z = f^T Q f, f = kron of (cos,sin) of 4 half-angles;
    Q = V diag(lam) V^T -> z = sum_i lam_i (V^T f)_i^2: TensorE
    block-diag matmuls in feature-major layout via PE transposes.
  * Slater E[n,m] = exp(i r_n.k_m); kvec 0 is (0,0) -> column 0 all-ones
    -> LU step 0 is a row subtraction; then batched no-pivot LU on 7x7
    via stride-0 broadcast APs (8192 matrices/core).
  * logdet = 0.5*Ln(|det|^2) + i*atan2(Im,Re) per spin (matches
    log(sign)+logabs of slogdet; no branch-cut wrapping).

Layouts (per core, 4096 walkers):
  particle-major planes [128, 512]: particle m = part*512 + col.
  matrix (w,s) -> part p = (w*16+8*s)//512, chunk tc = ((w*16+8*s)%512)//8
    i.e. w = p*32 + tc//2, s = tc%2; its 8 particles are columns
    tc*8..tc*8+7 of partition p.  Walker w -> (p, tw=tc//2).
  Jastrow & output use the same walker mapping (p, tw).

Scheduling notes (engines have in-order queues; emission order matters):
  * per half: both q PE-feed groups, then that half's phasor/LU --
    LU0 overlaps hf1's PE/ACT feed (measured 18us faster than emitting
    all four feed groups before any phasor work).
  * jastrow differences/abs-chain fill the early DVE window; sqrt +
    jastrow polynomial run on the tail-idle ACT (Identity scale/bias
    passes) + Pool (tensor muls).
  * lambda-contraction matmuls use bf16 gsq weights (fp32 weights cost
    2 half-rate PE passes each).
  * LU rank-1 updates: lre/lim expanded to packed bf16 [t,r,r] tiles by
    ACT copies so the DVE products run in the 2x (2-byte) mode; the A
    accumulation stays f32.  Pivots stay on the A diagonal (never
    overwritten); logdet = sum Ln|p_k|^2 + i*sum atan2 over the 7 pivots,
    batched wide at the end, phase wrapped to (-pi, pi].
  * output written interleaved [128,32,2] -> one contiguous DMA (the
    strided re/im pair DMA cost ~70us in 4-byte descriptors).
"""
import sys
import numpy as np

sys.path.insert(0, "/opt/trn_rl_repo")

from concourse import bass, mybir, tile  # noqa: E402
from concourse.bass_utils import run_bass_kernel_spmd  # noqa: E402

F32 = mybir.dt.float32
BF16 = mybir.dt.bfloat16
ALU = mybir.AluOpType
ACT = mybir.ActivationFunctionType
AXL = mybir.AxisListType

NCORES = 8
B = 32768
BLOC = B // NCORES          # 4096 walkers/core
NW = BLOC // 128            # 32 walkers per partition
M = BLOC * 16               # 65536 particles/core
MC = M // 128               # 512 particle columns
MCH = MC // 2               # pqc/LU half: 256 cols = 32 mats/part
BSH = 32                    # matrices per partition per half
JCH = 2
JB = NW // JCH              # 16 walkers per jastrow chunk
PIH = float(np.pi / 2)
PI = float(np.pi)

C_JP = 0
C_SC = [6, 18]
C_BC = [10, 22]
C_BSN = [14, 26]
C_KX = 32
C_KY = 39
C_MHALF = 46
C_PIHC = 47
C_MONE = 48
C_TPI = 49
C_MTPI = 50
C_HALF = 51
C_MTWO = 52
C_ONE = 53
NCONS = 54
WQ = 144                    # per-q weights: W_both 128 (rank-8, j-fused), RL_both 16
NWV = 2 * WQ


def _host_precompute(inputs):
    def rz(t):
        e = np.exp(-0.5j * t)
        return np.diag([e, np.conj(e)])

    def ry(t):
        c, s = np.cos(0.5 * t), np.sin(0.5 * t)
        return np.array([[c, -s], [s, c]], complex)

    def euler(p):
        return rz(p[2]) @ ry(p[1]) @ rz(p[0])

    def entangler(t):
        I4 = np.eye(4, dtype=complex)
        dzz = np.array([1., -1., -1., 1.])
        XX = np.array([[0, 0, 0, 1], [0, 0, 1, 0], [0, 1, 0, 0],
                       [1, 0, 0, 0]], complex)
        YY = np.array([[0, 0, 0, -1], [0, 0, 1, 0], [0, 1, 0, 0],
                       [-1, 0, 0, 0]], complex)
        rzz = lambda a: np.diag(np.exp(-0.5j * a * dzz))
        rxx = np.cos(0.5 * t[1]) * I4 - 1j * np.sin(0.5 * t[1]) * XX
        ryy = np.cos(0.5 * t[2]) * I4 - 1j * np.sin(0.5 * t[2]) * YY
        return rzz(t[3]) @ ryy @ rxx @ rzz(t[0])

    cons = np.zeros((128, NCONS), np.float32)
    wv = np.zeros((128, NWV), np.float32)
    cons[:, C_JP:C_JP + 6] = np.asarray(inputs["jastrow_param"], np.float64)
    X = np.array([[0, 1], [1, 0]], complex)
    I2 = np.eye(2, dtype=complex)
    for q in range(2):
        sq = np.asarray(inputs["param_single_qubit"][q], np.float64)
        tq = np.asarray(inputs["param_two_qubit"][q], np.float64)
        enc = np.asarray(inputs["param_encoding"][q], np.float64)
        enc_b = np.asarray(inputs["param_encoding_bias"][q], np.float64)
        U1 = (np.kron(euler(sq[0, 2]), euler(sq[0, 3])) @ entangler(tq[0])
              @ np.kron(euler(sq[0, 0]), euler(sq[0, 1])))
        U2 = (np.kron(euler(sq[1, 2]), euler(sq[1, 3])) @ entangler(tq[1])
              @ np.kron(euler(sq[1, 0]), euler(sq[1, 1])))
        D1 = np.diag([1, -1j, -1j, -1]).astype(complex)
        M1 = U1 @ D1
        KS = [np.kron(I2, I2), np.kron(I2, -1j * X),
              np.kron(-1j * X, I2), np.kron(-1j * X, -1j * X)]
        T = np.zeros((4, 16), complex)
        for i1 in range(2):
            for i2 in range(2):
                e12 = np.zeros(4)
                e12[2 * i1 + i2] = 1.0
                base = M1 @ e12
                for i3 in range(2):
                    for i4 in range(2):
                        T[:, 8 * i1 + 4 * i2 + 2 * i3 + i4] = \
                            U2 @ KS[2 * i3 + i4] @ base
        Z0 = np.diag([1., 1., -1., -1.]).astype(complex)
        Z1 = np.diag([1., -1., 1., -1.]).astype(complex)
        # Q has exact rank 8; keep the 8 nonzero eigenpairs and fuse the
        # two observables (j) into one weight block: out part = (j,a2,c).
        Wb = np.zeros((128, 128), np.float32)
        RLb = np.zeros((128, 16), np.float32)
        for j, Z in enumerate((Z0, Z1)):
            Q = np.real(T.conj().T @ Z @ T)
            Q = 0.5 * (Q + Q.T)
            lam, V = np.linalg.eigh(Q)
            idx8 = np.argsort(np.abs(lam))[-8:]
            V8 = V[:, idx8].astype(np.float32)
            lam8 = lam[idx8].astype(np.float32)
            for c in range(8):
                rows = np.arange(16) * 8 + c
                colsb = j * 64 + np.arange(8) * 8 + c
                Wb[np.ix_(rows, colsb)] = V8
                RLb[colsb, j * 8 + c] = lam8
        wv[:, q * WQ: q * WQ + 128] = Wb
        wv[:, q * WQ + 128: q * WQ + 144] = RLb
        scale = np.array([enc[0, 0], enc[0, 1], enc[1, 0], enc[1, 1]]) * np.pi
        bias = 0.5 * np.array([enc_b[0, 0], enc_b[0, 1],
                               enc_b[1, 0], enc_b[1, 1]])
        cons[:, C_SC[q]:C_SC[q] + 4] = scale
        cons[:, C_BC[q]:C_BC[q] + 4] = bias + PIH
        cons[:, C_BSN[q]:C_BSN[q] + 4] = bias
    kv = np.asarray(inputs["kvecs"], np.float64)
    assert abs(kv[0]).max() < 1e-6, "kernel assumes kvecs[0] == 0"
    cons[:, C_KX:C_KX + 7] = kv[1:8, 0]
    cons[:, C_KY:C_KY + 7] = kv[1:8, 1]
    cons[:, C_MHALF] = -0.5
    cons[:, C_PIHC] = PIH
    cons[:, C_MONE] = -1.0
    cons[:, C_TPI] = 2 * PI
    cons[:, C_MTPI] = -2 * PI
    cons[:, C_HALF] = 0.5
    cons[:, C_MTWO] = -2.0
    cons[:, C_ONE] = 1.0
    pc = np.asarray(inputs["param_classical"], np.float64)
    pcv = np.zeros((128, 4), np.float32)
    pcv[:, 0:2] = pc[0]
    pcv[:, 2:4] = pc[1]
    ident = np.eye(128, dtype=np.float32)
    return cons, wv, ident, pcv


def build(gpat=None, debug=False, loop_n=0, trace_sim=False):
    if gpat is None:
        gpat = ((-1, 0), (0, -1), (0, 1), (1, 0),
                (-1, -1), (-1, 1), (1, -1))
    nc = bass.Bass()
    x_d = nc.declare_dram_parameter("x", [BLOC, 32], F32, isOutput=False)
    NKC = NCONS + NWV + 128 + 4
    kc_d = nc.declare_dram_parameter("kc", [128, NKC], F32, isOutput=False)
    out_d = nc.declare_dram_parameter("out", [BLOC, 2], F32, isOutput=True)
    if debug:
        dbg_jas = nc.declare_dram_parameter("dbg_jas", [128, NW], F32,
                                            isOutput=True)
        dbg_z = nc.declare_dram_parameter("dbg_z", [128, 4, MC], F32,
                                          isOutput=True)
        dbg_E = nc.declare_dram_parameter("dbg_E", [128, 2, BSH, 8, 7], F32,
                                          isOutput=True)
        dbg_det = nc.declare_dram_parameter("dbg_det", [128, 4, BSH], F32,
                                            isOutput=True)

    xflat = x_d[:].rearrange("b c -> (b c)")

    _tc_holder = {}
    with tile.TileContext(nc, trace_sim=trace_sim) as tc:
        _tc_holder['tc'] = tc
        with (
            tc.tile_pool(name="const", bufs=1) as cpool,
            tc.tile_pool(name="pers", bufs=1) as pers,
            tc.tile_pool(name="jt", bufs=1) as jt,
            tc.tile_pool(name="pt", bufs=1) as pt,
            tc.tile_pool(name="gt", bufs=4) as gt,
            tc.tile_pool(name="et", bufs=2) as et,
            tc.tile_pool(name="lt", bufs=2) as lt,
            tc.tile_pool(name="st", bufs=2) as st,
            tc.tile_pool(name="ps_t", bufs=3, space="PSUM") as ps_t,
            tc.tile_pool(name="ps_g", bufs=3, space="PSUM") as ps_g,
            tc.tile_pool(name="ps_w", bufs=1, space="PSUM") as ps_w,
        ):
            kc = cpool.tile([128, NKC], F32, tag="kc")
            nc.sync.dma_start(kc[:], kc_d[:])
            cons = kc[:, 0:NCONS]
            wvt = kc[:, NCONS:NCONS + NWV]
            ident = kc[:, NCONS + NWV:NCONS + NWV + 128]
            pcv = kc[:, NCONS + NWV + 128:NCONS + NWV + 132]

            wvb = cpool.tile([128, NWV], BF16, tag="wvb")
            nc.scalar.copy(wvb[:], kc[:, NCONS:NCONS + NWV])
            identb = cpool.tile([128, 128], BF16, tag="identb")
            nc.scalar.copy(identb[:], ident)

            def cc(i):
                return cons[:, i:i + 1]

            # =============== Jastrow (walker (p,tw) mapping) ===========
            xin = pers.tile([128, NW, 32], F32, tag="xin")
            xdv = xflat.rearrange("(p tw c) -> p tw c", p=128, tw=NW, c=32)
            # split the input DMA by walker halves: half-0 compute
            # (jastrow chunk 0, hf0 trig) starts after the first half lands
            nc.sync.dma_start(xin[:, 0:NW // 2, :], xdv[:, 0:NW // 2, :])
            nc.sync.dma_start(xin[:, NW // 2:NW, :], xdv[:, NW // 2:NW, :])
            xall = xin[:].rearrange("p tw c -> p (tw c)").rearrange(
                "p (cc d) -> p cc d", d=2)
            xsep = pers.tile([128, 2, MC], F32, tag="xsep")
            for hx in range(2):
                hs = slice(hx * MCH, (hx + 1) * MCH)
                nc.vector.tensor_copy(xsep[:, 0, hs], xall[:, hs, 0])
                nc.vector.tensor_copy(xsep[:, 1, hs], xall[:, hs, 1])
            # ScalarE warm-ups: observe each DMA queue once so no real ACT
            # instruction ever needs two semaphore waits (ISA limit is 1).
            wsc1 = cpool.tile([128, 1], F32, tag="wsc1")
            wsc2 = cpool.tile([128, 1], F32, tag="wsc2")
            nc.scalar.activation(wsc1[:], kc[:, 0:1], ACT.Copy)
            nc.scalar.activation(wsc2[:], xin[:, 0, 0:1], ACT.Copy)
            wps = ps_w.tile([8, 8], F32, tag="wps")
            nc.tensor.transpose(wps[:], ident[0:8, 0:8], ident[0:8, 0:8])
            import contextlib
            _lcm = tc.For_i(0, loop_n, 1) if loop_n else \
                contextlib.nullcontext()
            _lcm.__enter__()
            # =============== PQC backflow -> zplm[q][:,j,:] ============
            zplm = [pers.tile([128, 2, MC], F32, tag=f"zplm{q}",
                              name=f"zplm{q}") for q in range(2)]
            zpl = [[zplm[q][:, j] for j in range(2)] for q in range(2)]


            # PQC frontend for all (hf,q) first: trig (ACT) + feature
            # products (DVE) so the PE transpose/matmul pipeline starts
            # immediately; jastrow then fills ACT/DVE/Pool gaps.
            fts = {}

            def _frontend(hf):
                c0 = hf * MCH
                for q in range(2):
                    trig = pt.tile([128, 8, MCH], BF16, tag="trig")
                    for j in range(4):
                        coord = xsep[:, j % 2, c0:c0 + MCH]
                        nc.scalar.activation(trig[:, 2 * j, :], coord,
                                             ACT.Sin, bias=cc(C_BC[q] + j),
                                             scale=cc(C_SC[q] + j))
                        nc.scalar.activation(trig[:, 2 * j + 1, :], coord,
                                             ACT.Sin, bias=cc(C_BSN[q] + j),
                                             scale=cc(C_SC[q] + j))
                    u = pt.tile([128, 2, 2, MCH], BF16, tag="u")
                    nc.vector.tensor_mul(
                        u[:],
                        trig[:, 0:2, :].unsqueeze(2).broadcast_to(
                            (128, 2, 2, MCH)),
                        trig[:, 2:4, :].unsqueeze(1).broadcast_to(
                            (128, 2, 2, MCH)))
                    v = pt.tile([128, 2, 2, MCH], BF16, tag="v")
                    nc.vector.tensor_mul(
                        v[:],
                        trig[:, 4:6, :].unsqueeze(2).broadcast_to(
                            (128, 2, 2, MCH)),
                        trig[:, 6:8, :].unsqueeze(1).broadcast_to(
                            (128, 2, 2, MCH)))
                    f = pt.tile([128, MCH // 8, 16, 8], BF16,
                                tag=f"f{hf}{q}")
                    fo = f[:].rearrange("p t (a b) c -> p a b t c", a=4)
                    nc.vector.tensor_mul(
                        fo,
                        u[:].rearrange("p a b (t c) -> p (a b) t c", c=8)
                            .unsqueeze(2).broadcast_to(
                                (128, 4, 4, MCH // 8, 8)),
                        v[:].rearrange("p a b (t c) -> p (a b) t c", c=8)
                            .unsqueeze(1).broadcast_to(
                                (128, 4, 4, MCH // 8, 8)))
                    fts[(hf, q)] = f

            _frontend(0)

            # ===== Jastrow front: fused pair differences only (DVE).
            # The |d|/min-image/poly chain runs post-feed on the
            # tail-idle ACT (see below).
            jas = pers.tile([128, NW], F32, tag="jas")
            js = pers.tile([128, NW, 120], BF16, tag="js")
            jd = jt.tile([128, NW, 240], BF16, tag="jd")
            off = 0
            for o in range(1, 16):
                Lg = 32 - 2 * o
                nc.vector.tensor_sub(jd[:, :, off:off + Lg],
                                     xin[:, :, 0:Lg],
                                     xin[:, :, 2 * o:32])
                off += Lg
            outri = pers.tile([128, NW, 2], F32, tag="outri")
            prs = pers.tile([128, 2, BSH, 7], F32, tag="prs")
            pis = pers.tile([128, 2, BSH, 7], F32, tag="pis")
            # PE feed for all four (hf,q) groups first — keeps the
            # in-order ACT queue free of phasor ops that would stall it.
            for hf in range(2):
                for q in range(2):
                    f = fts[(hf, q)]
                    for gl in range(8):
                        grp = hf * 8 + gl
                        ftp = ps_t.tile([128, 512], BF16, tag="ftp")
                        for gi in range(4):
                            ti = gl * 4 + gi
                            nc.tensor.transpose(
                                ftp[:, gi * 128:(gi + 1) * 128],
                                f[:, ti].rearrange("p a c -> p (a c)"),
                                identb[:])
                        ftr = gt.tile([128, 512], BF16, tag="ftr")
                        nc.scalar.copy(ftr[:], ftp[:])
                        gp = ps_g.tile([128, 512], F32, tag="gp")
                        ztp = ps_w.tile([128, 4, 2, 8], F32, tag="ztp")
                        nc.tensor.matmul(
                            gp[:],
                            wvb[:, q * WQ:q * WQ + 128],
                            ftr[:])
                        gsq = gt.tile([128, 512], BF16, tag="gsq")
                        nc.scalar.activation(gsq[:], gp[:], ACT.Square)
                        for gi in range(4):
                            nc.tensor.matmul(
                                ztp[:, gi, :, :],
                                gsq[:, gi * 128:(gi + 1) * 128],
                                wvb[:, q * WQ + 128:q * WQ + 144])
                        if hf == 0:
                            nc.vector.tensor_copy(
                                zplm[q][:, :, grp * 32:(grp + 1) * 32]
                                .rearrange("p j (a b) -> p j a b", a=4),
                                ztp[:].rearrange("p a j b -> p j a b"))
                        else:
                            nc.scalar.copy(
                                zplm[q][:, :, grp * 32:(grp + 1) * 32]
                                .rearrange("p j (a b) -> p j a b", a=4),
                                ztp[:].rearrange("p a j b -> p j a b"))

                if hf == 0:
                    _frontend(1)
                c0 = hf * MCH
                csl = slice(c0, c0 + MCH)
                # xc planes for this half
                xrh2 = et.tile([128, 2, MCH], F32, tag="xrh2",
                               name=f"xrh2{hf}", bufs=1)
                xih2 = et.tile([128, 2, MCH], F32, tag="xih2",
                               name=f"xih2{hf}", bufs=1)
                for dd in range(2):
                    nc.vector.scalar_tensor_tensor(
                        xrh2[:, dd, :], zpl[0][dd][:, csl],
                        pcv[:, dd:dd + 1],
                        xsep[:, dd, csl], ALU.mult, ALU.add)
                    nc.vector.tensor_scalar_mul(
                        xih2[:, dd, :], zpl[1][dd][:, csl],
                        pcv[:, 2 + dd:3 + dd])

                # range reduction, Sin/Exp and magnitude products all
                # fused across both coordinate dims (halves op counts)
                msk = et.tile([128, 2, MCH], F32, tag="emsk", bufs=1)
                u2t = et.tile([128, 2, MCH], F32, tag="eu2", bufs=1)
                v2t = et.tile([128, 2, MCH], F32, tag="ev2", bufs=1)
                nc.vector.tensor_scalar(msk[:], xrh2[:], 0.5, None,
                                        ALU.is_ge)
                nc.vector.tensor_sub(u2t[:], xrh2[:], msk[:])
                nc.vector.tensor_scalar(msk[:], u2t[:], 0.25, None,
                                        ALU.add)
                nc.vector.tensor_scalar(v2t[:], msk[:], 0.5, None,
                                        ALU.is_ge)
                nc.vector.tensor_sub(v2t[:], msk[:], v2t[:])
                trs = et.tile([128, 2, MCH], F32, tag="etrs",
                              name=f"etrs{hf}", bufs=1)
                trc = et.tile([128, 2, MCH], F32, tag="etrc",
                              name=f"etrc{hf}", bufs=1)
                nc.scalar.activation(trs[:], u2t[:], ACT.Sin,
                                     scale=cc(C_TPI))
                nc.scalar.activation(trc[:], v2t[:], ACT.Sin,
                                     scale=cc(C_TPI))
                mdp = et.tile([128, 2, MCH], F32, tag="emdp",
                              name=f"emdp{hf}", bufs=1)
                mdm = et.tile([128, 2, MCH], F32, tag="emdm",
                              name=f"emdm{hf}", bufs=1)
                nc.scalar.activation(mdp[:], xih2[:], ACT.Exp,
                                     scale=cc(C_MTPI))
                nc.scalar.activation(mdm[:], xih2[:], ACT.Exp,
                                     scale=cc(C_TPI))
                frp = et.tile([128, 2, MCH], F32, tag="efrp",
                              name=f"efrp{hf}", bufs=1)
                fip = et.tile([128, 2, MCH], F32, tag="efip",
                              name=f"efip{hf}", bufs=1)
                frm = et.tile([128, 2, MCH], F32, tag="efrm",
                              name=f"efrm{hf}", bufs=1)
                fim = et.tile([128, 2, MCH], F32, tag="efim",
                              name=f"efim{hf}", bufs=1)
                nc.vector.tensor_mul(frp[:], mdp[:], trc[:])
                nc.vector.tensor_mul(fip[:], mdp[:], trs[:])
                nc.vector.tensor_mul(frm[:], mdm[:], trc[:])
                nc.vector.tensor_mul(fim[:], mdm[:], trs[:])
                names = {}
                for d2 in range(2):
                    names[(d2, 1)] = (frp[:, d2], fip[:, d2], 1)
                    names[(d2, -1)] = (frm[:, d2], fim[:, d2], -1)
                cols = []
                for (gx, gy) in gpat:
                    if gx != 0 and gy == 0:
                        cols.append(names[(0, gx)])
                    elif gx == 0 and gy != 0:
                        cols.append(names[(1, gy)])
                    else:
                        xr_, xi_, sx = names[(0, gx)]
                        yr_, yi_, sy = names[(1, gy)]
                        pre = et.tile([128, MCH], F32, tag=f"pr{gx}{gy}",
                                      name=f"pr{gx}{gy}{hf}", bufs=1)
                        pim = et.tile([128, MCH], F32, tag=f"pi{gx}{gy}",
                                      name=f"pi{gx}{gy}{hf}", bufs=1)
                        t1_ = et.tile([128, MCH], F32, tag="ept1", bufs=1)
                        t2_ = et.tile([128, MCH], F32, tag="ept2", bufs=1)
                        nc.gpsimd.tensor_mul(t1_[:], xr_, yr_)
                        nc.vector.tensor_mul(t2_[:], xi_, yi_)
                        nc.vector.tensor_tensor(
                            pre[:], t1_[:], t2_[:],
                            ALU.subtract if sx * sy > 0 else ALU.add)
                        nc.gpsimd.tensor_mul(t1_[:], xi_, yr_)
                        nc.vector.tensor_mul(t2_[:], xr_, yi_)
                        if sx > 0 and sy > 0:
                            nc.vector.tensor_add(pim[:], t1_[:], t2_[:])
                            isn = 1
                        elif sx < 0 and sy < 0:
                            nc.vector.tensor_add(pim[:], t1_[:], t2_[:])
                            isn = -1
                        elif sx > 0:
                            nc.vector.tensor_sub(pim[:], t1_[:], t2_[:])
                            isn = 1
                        else:
                            nc.vector.tensor_sub(pim[:], t2_[:], t1_[:])
                            isn = 1
                        cols.append((pre[:], pim[:], isn))

                # A-build (fused step-0 of the LU: col0 of E is all-ones)
                Arr = et.tile([128, BSH, 7, 7], F32, tag="Ar")
                Aii = et.tile([128, BSH, 7, 7], F32, tag="Ai")
                for j, (re, im, isn) in enumerate(cols):
                    rev = re.rearrange("p (t n) -> p t n", n=8)
                    imv = im.rearrange("p (t n) -> p t n", n=8)
                    # transposed build (S = A^T, det identical): row
                    # writes are packed on Pool instead of stride-7 cols
                    nc.gpsimd.tensor_sub(
                        Arr[:, :, j, :], rev[:, :, 1:8],
                        rev[:, :, 0:1].broadcast_to((128, BSH, 7)))
                    if isn > 0:
                        nc.gpsimd.tensor_sub(
                            Aii[:, :, j, :], imv[:, :, 1:8],
                            imv[:, :, 0:1].broadcast_to((128, BSH, 7)))
                    else:
                        nc.gpsimd.tensor_sub(
                            Aii[:, :, j, :],
                            imv[:, :, 0:1].broadcast_to((128, BSH, 7)),
                            imv[:, :, 1:8])
                SPL = 23
                for k in range(6):
                    r = 6 - k
                    pr = Arr[:, :, k, k]
                    pi = Aii[:, :, k, k]
                    # raw columns expanded on ACT (the only strided-col
                    # read); row' = conj(P)/|P|^2 * row reads PACKED rows.
                    lrx = lt.tile([128, BSH, 6, 6], BF16, tag="lrx", bufs=1)
                    lix = lt.tile([128, BSH, 6, 6], BF16, tag="lix", bufs=1)
                    nc.scalar.copy(
                        lrx[:, :, 0:r, 0:r],
                        Arr[:, :, k + 1:7, k:k + 1].broadcast_to(
                            (128, BSH, r, r)))
                    nc.scalar.copy(
                        lix[:, :, 0:r, 0:r],
                        Aii[:, :, k + 1:7, k:k + 1].broadcast_to(
                            (128, BSH, r, r)))
                    t1 = st.tile([128, BSH], F32, tag="lt1", bufs=1)
                    t2 = st.tile([128, BSH], F32, tag="lt2", bufs=1)
                    nc.vector.tensor_mul(t1[:], pr, pr)
                    nc.vector.tensor_mul(t2[:], pi, pi)
                    nc.vector.tensor_add(t1[:], t1[:], t2[:])
                    rinv = st.tile([128, BSH], F32, tag="lrinv", bufs=1)
                    nc.vector.reciprocal(rinv[:], t1[:])
                    asc = st.tile([128, BSH], F32, tag="lasc", bufs=1)
                    bsc = st.tile([128, BSH], F32, tag="lbsc", bufs=1)
                    nc.vector.tensor_mul(asc[:], pr, rinv[:])
                    nc.vector.tensor_mul(bsc[:], pi, rinv[:])
                    rowr = Arr[:, :, k, k + 1:7]
                    rowi = Aii[:, :, k, k + 1:7]
                    abc = asc[:].unsqueeze(2).broadcast_to((128, BSH, r))
                    bbc = bsc[:].unsqueeze(2).broadcast_to((128, BSH, r))
                    m1 = st.tile([128, BSH, 6], F32, tag="lu1", bufs=1)
                    m2 = st.tile([128, BSH, 6], F32, tag="lu2", bufs=1)
                    rre = st.tile([128, BSH, 6], BF16, tag="llre", bufs=1)
                    rim = st.tile([128, BSH, 6], BF16, tag="llim", bufs=1)
                    m1v, m2v = m1[:, :, 0:r], m2[:, :, 0:r]
                    rrev, rimv = rre[:, :, 0:r], rim[:, :, 0:r]
                    nc.vector.tensor_mul(m1v, rowr, abc)
                    nc.vector.tensor_mul(m2v, rowi, bbc)
                    nc.vector.tensor_add(rrev, m1v, m2v)
                    nc.vector.tensor_mul(m1v, rowi, abc)
                    nc.vector.tensor_mul(m2v, rowr, bbc)
                    nc.vector.tensor_sub(rimv, m1v, m2v)
                    w1 = lt.tile([128, BSH, 6, 6], BF16, tag="lw1", bufs=1)
                    w2 = lt.tile([128, BSH, 6, 6], BF16, tag="lw2", bufs=1)
                    w3 = lt.tile([128, BSH, 6, 6], BF16, tag="lw3", bufs=1)
                    w4 = lt.tile([128, BSH, 6, 6], BF16, tag="lw4", bufs=1)
                    lreb = lrx[:, :, 0:r, 0:r]
                    limb = lix[:, :, 0:r, 0:r]
                    rre_bc = rrev.unsqueeze(2).broadcast_to(
                        (128, BSH, r, r))
                    rim_bc = rimv.unsqueeze(2).broadcast_to(
                        (128, BSH, r, r))
                    w1v = w1[:, :, 0:r, 0:r]
                    w2v = w2[:, :, 0:r, 0:r]
                    w3v = w3[:, :, 0:r, 0:r]
                    w4v = w4[:, :, 0:r, 0:r]
                    nc.vector.tensor_mul(w1v, lreb, rre_bc)
                    nc.vector.tensor_mul(w3v, lreb, rim_bc)
                    nc.vector.tensor_mul(w2v, limb, rim_bc)
                    nc.vector.tensor_sub(w1v, w1v, w2v)
                    nc.vector.tensor_sub(Arr[:, :, k + 1:7, k + 1:7],
                                         Arr[:, :, k + 1:7, k + 1:7], w1v)
                    nc.vector.tensor_mul(w4v, limb, rre_bc)
                    nc.vector.tensor_add(w3v, w3v, w4v)
                    nc.vector.tensor_sub(Aii[:, :, k + 1:7, k + 1:7],
                                         Aii[:, :, k + 1:7, k + 1:7], w3v)
                for k in range(7):
                    nc.vector.tensor_copy(prs[:, hf, :, k], Arr[:, :, k, k])
                    nc.vector.tensor_copy(pis[:, hf, :, k], Aii[:, :, k, k])

            # jastrow |d| -> min-image -> jv chain: ACT is idle during
            # the LU tail; only 2 DVE muls.  jv = A*(1-2*A*adm)^2 with
            # A = adm^2, via Square(-2C+1) fusing u and u^2.
            ja1 = jt.tile([128, NW, 240], BF16, tag="ja1")
            jA = jt.tile([128, NW, 240], BF16, tag="jA")
            nc.scalar.activation(ja1[:], jd[:], ACT.Abs)
            nc.scalar.activation(ja1[:], ja1[:], ACT.Abs,
                                 bias=cc(C_MHALF))
            nc.scalar.activation(jA[:], ja1[:], ACT.Square,
                                 scale=cc(C_MONE), bias=cc(C_HALF))
            nc.scalar.activation(ja1[:], ja1[:], ACT.Identity,
                                 scale=cc(C_MONE), bias=cc(C_HALF))
            nc.vector.tensor_mul(jd[:], jA[:], ja1[:])
            nc.scalar.activation(jd[:], jd[:], ACT.Square,
                                 scale=cc(C_MTWO), bias=cc(C_ONE))
            nc.vector.tensor_mul(jd[:], jA[:], jd[:])
            jdv = jd[:].rearrange("p b (q t) -> p b q t", t=2)
            nc.gpsimd.tensor_add(js[:], jdv[:, :, :, 0], jdv[:, :, :, 1])
            # jastrow sqrt + polynomial on tail-idle ACT/Pool
            s = js[:]
            tv = ja1[:, :, 0:120]
            p1 = ja1[:, :, 120:240]
            p2 = jA[:, :, 0:120]
            nc.scalar.activation(tv, s, ACT.Sqrt)
            nc.scalar.activation(p1, s, ACT.Identity,
                                 bias=cc(C_JP + 2), scale=cc(C_JP + 4))
            nc.gpsimd.tensor_tensor(p1, p1, s, ALU.mult)
            nc.scalar.activation(p1, p1, ACT.Identity,
                                 bias=cc(C_JP + 0))
            nc.scalar.activation(p2, s, ACT.Identity,
                                 bias=cc(C_JP + 3), scale=cc(C_JP + 5))
            nc.gpsimd.tensor_tensor(p2, p2, s, ALU.mult)
            nc.scalar.activation(p2, p2, ACT.Identity,
                                 bias=cc(C_JP + 1))
            nc.gpsimd.tensor_tensor(p1, p1, tv, ALU.mult)
            nc.gpsimd.tensor_tensor(p2, p2, s, ALU.mult)
            nc.vector.tensor_add(p1, p1, p2)
            nc.vector.tensor_reduce(jas[:], p1, axis=AXL.X, op=ALU.add)
            if debug:
                nc.sync.dma_start(dbg_jas[:], jas[:])

            # ===== batched logdet from the 2*BSH*7 saved pivots =====
            # |det|^2 = prod |p_k|^2 ; arg(det) = sum atan2(pi_k, pr_k)
            # wrapped to (-pi, pi].
            pv = prs[:].rearrange("p h t k -> p (h t k)")
            iv = pis[:].rearrange("p h t k -> p (h t k)")
            NP = 2 * BSH * 7
            den = lt.tile([128, NP], F32, tag="fden", bufs=1)
            tmp = lt.tile([128, NP], F32, tag="ftmp", bufs=1)
            nc.gpsimd.tensor_mul(den[:], pv, pv)
            nc.gpsimd.tensor_mul(tmp[:], iv, iv)
            nc.gpsimd.tensor_add(den[:], den[:], tmp[:])
            lnd = lt.tile([128, NP], F32, tag="ftmp", bufs=1)
            nc.scalar.activation(lnd[:], den[:], ACT.Ln)
            rp = lt.tile([128, NP], F32, tag="frp", bufs=1)
            nc.vector.reciprocal(rp[:], pv)
            nc.vector.tensor_mul(rp[:], iv, rp[:])
            at = lt.tile([128, NP], F32, tag="fden", bufs=1)
            nc.scalar.activation(at[:], rp[:], ACT.Arctan)
            m1 = lt.tile([128, NP], F32, tag="frp", bufs=1)
            m2 = lt.tile([128, NP], F32, tag="fm2", bufs=1)
            nc.vector.tensor_scalar(m1[:], pv, 0.0, None, ALU.is_lt)
            nc.vector.tensor_scalar(m2[:], iv, 0.0, None, ALU.is_ge)
            nc.vector.scalar_tensor_tensor(m2[:], m2[:], 2.0, m1[:],
                                           ALU.mult, ALU.mult)
            nc.vector.tensor_sub(m2[:], m2[:], m1[:])
            nc.vector.scalar_tensor_tensor(at[:], m2[:], PI, at[:],
                                           ALU.mult, ALU.add)
            labs = st.tile([128, 2, BSH], F32, tag="flabs")
            args = st.tile([128, 2, BSH], F32, tag="fargs")
            nc.vector.tensor_reduce(
                labs[:], lnd[:].rearrange("p (h t k) -> p h t k",
                                          h=2, k=7),
                axis=AXL.X, op=ALU.add)
            nc.vector.tensor_reduce(
                args[:], at[:].rearrange("p (h t k) -> p h t k",
                                         h=2, k=7),
                axis=AXL.X, op=ALU.add)
            wm = st.tile([128, 2, BSH], F32, tag="fwm")
            for _ in range(3):
                nc.vector.tensor_scalar(wm[:], args[:], PI, None, ALU.is_ge)
                nc.vector.scalar_tensor_tensor(args[:], wm[:], -2 * PI,
                                               args[:], ALU.mult, ALU.add)
                nc.vector.tensor_scalar(wm[:], args[:], -PI, None,
                                        ALU.is_lt)
                nc.vector.scalar_tensor_tensor(args[:], wm[:], 2 * PI,
                                               args[:], ALU.mult, ALU.add)
            for hf in range(2):
                labv = labs[:, hf].rearrange("p (tw s) -> p tw s", s=2)
                argv = args[:, hf].rearrange("p (tw s) -> p tw s", s=2)
                wsl = slice(hf * (NW // 2), (hf + 1) * (NW // 2))
                lsum = st.tile([128, NW // 2], F32, tag="olsum")
                nc.vector.tensor_add(lsum[:], labv[:, :, 0], labv[:, :, 1])
                nc.vector.scalar_tensor_tensor(
                    outri[:, wsl, 0], lsum[:], 0.5, jas[:, wsl],
                    ALU.mult, ALU.add)
                nc.vector.tensor_add(outri[:, wsl, 1], argv[:, :, 0],
                                     argv[:, :, 1])

            ov = out_d[:].rearrange("(p tw) r -> p tw r", p=128)
            nc.sync.dma_start(ov[:], outri[:])
            _lcm.__exit__(None, None, None)
    _legalize_waits(nc)
    if trace_sim:
        return nc, _tc_holder['tc']
    return nc


def _legalize_waits(nc):
    """This walrus build allows only ONE sync wait per instruction;
    Tile emits several. Split extras onto EventSemaphore nops."""
    n = 0
    for fn in nc.m.functions:
        for b in fn.blocks:
            out = []
            for ins in b.instructions:
                si = ins.sync_info
                if si is not None and si.on_wait and len(si.on_wait) > 1:
                    waits = list(si.on_wait)
                    for i, w in enumerate(waits[:-1]):
                        out.append(mybir.InstEventSemaphore(
                            name=f"WSPLIT{n}-{ins.name}",
                            engine=ins.engine,
                            sync_info=mybir.SyncInfo(on_wait=[w],
                                                     on_update=[]),
                            ins=[], outs=[], debug=ins.debug))
                        n += 1
                    ins.sync_info = mybir.SyncInfo(
                        on_wait=[waits[-1]],
                        on_update=list(si.on_update or []))
                out.append(ins)
            b.instructions = out
    return n


_CACHE = {}


def make_in_maps(inputs):
    cons, wv, ident, pcv = _host_precompute(inputs)
    kc = np.concatenate([cons, wv, ident, pcv], axis=1)
    x = np.ascontiguousarray(np.asarray(inputs["x"], np.float32))
    return [{
        "x": x[c * BLOC:(c + 1) * BLOC], "kc": kc,
    } for c in range(NCORES)]


def kernel(**inputs):
    if "nc" not in _CACHE:
        _CACHE["nc"] = build()
    nc = _CACHE["nc"]
    in_maps = make_in_maps(inputs)
    res = run_bass_kernel_spmd(nc, in_maps, core_ids=list(range(NCORES)))
    outs = [res.results[c]["out"] for c in range(NCORES)]
    full = np.concatenate(outs, axis=0)
    return (full[:, 0] + 1j * full[:, 1]).astype(np.complex64)

